# revision 1
# baseline (speedup 1.0000x reference)
"""BiLSTM-CRF (Viterbi decode) Trainium2 Bass kernel, 8-core data-parallel.

Full inputs in, full outputs out. Batch (64) is sharded 8 ways; each core runs:
  embedding gather -> input matmuls (gx = x @ Wih^T + b) -> 256-step fused
  fwd+bwd LSTM recurrence -> fc emissions -> Viterbi scan -> batched
  backpointer extraction -> backtrace.

Layout convention on device ("version B"): gate/hidden dims live on SBUF
partitions, batch on the free dim, so ACT/DVE use all 128 lanes.
"""

import os
import sys
import types

for _p in ('/opt/trn_rl_repo', '/root/.axon_site'):
    if _p not in sys.path:
        sys.path.insert(0, _p)

import numpy as np
import ml_dtypes

# ---- NTFF profile hook (lets run_bass_kernel_spmd(trace=True) return timings
# under axon; harmless if already registered or unavailable) ----
def _install_ntff_hook():
    try:
        import antenv
        if 'antenv.axon_hooks' in sys.modules:
            return
        from trn_agent_boot.trn_boot import _ntff_profile_via_ctypes
        m = types.ModuleType('antenv.axon_hooks')
        m._hook = _ntff_profile_via_ctypes('/opt/axon/libaxon_pjrt.so')
        m.get_axon_ntff_profile_hook = lambda: m._hook
        m.set_axon_ntff_profile_hook = lambda h: setattr(m, '_hook', h)
        sys.modules['antenv.axon_hooks'] = m
        antenv.axon_hooks = m
    except Exception:
        pass


_install_ntff_hook()

import concourse.bass as bass
import concourse.tile as tile
from concourse import bacc, mybir
from concourse.bass import IndirectOffsetOnAxis
from concourse.bass_utils import run_bass_kernel_spmd

F32 = mybir.dt.float32
BF16 = mybir.dt.bfloat16
I32 = mybir.dt.int32

# Problem dims (hardcoded per contract)
V, E, HS, T, B = 30000, 256, 512, 256, 64
H = HS // 2          # 256 per-direction hidden
G = 4 * H            # 1024 gate rows per direction
K = 10               # tags
NC_ = 8              # cores
BL = B // NC_        # 8 sequences per core
NBT = BL * T         # 2048 (b,t) columns per core
NSLOT = NBT // 128   # 16 gather slots

# Gate reorder: torch rows [i, f, g, o] -> device order [i, f, o, g]
# (sigmoid block = chunks 0..5, tanh block = chunks 6..7)
_PERM = np.concatenate([
    np.arange(0, 2 * H),          # i, f
    np.arange(3 * H, 4 * H),      # o
    np.arange(2 * H, 3 * H),      # g
])


def _bf(x):
    return np.ascontiguousarray(np.asarray(x, np.float32).astype(ml_dtypes.bfloat16))


def _f32(x):
    return np.ascontiguousarray(np.asarray(x, np.float32))


def _pack_w(wih, whh, bih, bhh):
    """Per direction: returns (w_ih[128, 2*8*128], w_hh[...], bias[128, 8]) in
    lhsT tile layout w[p, kc, mc, m] = W[perm[mc*128+m], kc*128+p]."""
    out = []
    for W in (wih, whh):
        Wp = np.asarray(W, np.float32)[_PERM]          # [G, Kdim]
        Kd = Wp.shape[1]
        t = Wp.reshape(8, 128, Kd // 128, 128)          # [mc, m, kc, p]
        t = np.transpose(t, (3, 2, 0, 1))               # [p, kc, mc, m]
        out.append(t.reshape(128, -1))
    b = (np.asarray(bih, np.float32) + np.asarray(bhh, np.float32))[_PERM]
    b = b.reshape(8, 128).T                             # [p, mc]
    return out[0], out[1], b


def _prep_core(inputs, core):
    """Host-side prep of all per-core device inputs."""
    s = slice(core * BL, (core + 1) * BL)
    inp = np.asarray(inputs['inp'])[s]        # [8, 256] int
    n = np.asarray(inputs['n'])[s].astype(np.int64)

    t_idx = np.arange(T)
    mask = t_idx[None, :] < n[:, None]
    rev = np.where(mask, n[:, None] - 1 - t_idx[None, :], t_idx[None, :])
    tok_rev = np.take_along_axis(inp, rev, axis=1)

    def idx_pack(tok):  # [8,256] -> [128, 16] slot layout (j = s*128+p, j=b*256+t)
        flat = np.asarray(tok, np.int64).reshape(-1)     # j = b*256+t
        return flat.reshape(NSLOT, 128).T.astype(np.int32).copy()

    wf = _pack_w(inputs['W_ih_f'], inputs['W_hh_f'], inputs['b_ih_f'], inputs['b_hh_f'])
    wb = _pack_w(inputs['W_ih_b'], inputs['W_hh_b'], inputs['b_ih_b'], inputs['b_hh_b'])
    w_ih = _bf(np.concatenate([wf[0], wb[0]], axis=1))   # [128, 2*2048]
    w_hh = _bf(np.concatenate([wf[1], wb[1]], axis=1))
    bias = _f32(np.concatenate([wf[2], wb[2]], axis=1))  # [128, 16] (d, mc)

    fcw = np.asarray(inputs['fc_w'], np.float32)         # [10, 512]
    fcw_t = fcw.T.reshape(4, 128, K).transpose(1, 0, 2).reshape(128, 4 * K)
    fcbR = np.tile(np.asarray(inputs['fc_b'], np.float32)[None, :], (128, 1))

    trans = np.asarray(inputs['transition'], np.float32)[:K, :K]  # [prev, cur]
    transR = np.tile(trans.T.reshape(1, K * K), (128, 1)).copy()  # [p, cur*10+prev]

    iotaD = np.tile((9.0 - np.arange(K, dtype=np.float32))[None, :], (128, 1))
    iotaK = np.tile(np.arange(K, dtype=np.float32)[None, :], (128, 1))

    # validT[p, h*8+b] = ((h*128+p)+1 < n_b); ivT[p, (h,b,k)] = k*(1-valid)
    tt = (np.arange(256).reshape(2, 128).T)[:, :, None]            # [p, h, 1]
    validT = (tt + 1 < n[None, None, :]).astype(np.float32)        # [p, h, b]
    ivT = (1.0 - validT)[:, :, :, None] * np.arange(K, dtype=np.float32)[None, None, None, :]

    endSel = np.zeros((128, T), np.float32)
    endSel[np.arange(BL), (n - 1)] = 1.0
    maskBT = np.zeros((128, T), np.float32)
    maskBT[:BL] = mask.astype(np.float32)

    # hb re-reversal gather rows: out col j=(b,t) <- hb_dram row b*256 + scan_idx
    scan_idx = np.where(mask, n[:, None] - 1 - t_idx[None, :], t_idx[None, :])
    hb_rows = ((scan_idx // 16) * 128 + (scan_idx % 16) * 8
               + np.arange(BL)[:, None]).reshape(-1)
    hb_off = hb_rows.reshape(NSLOT, 128).T.astype(np.int32).copy()

    return {
        'emb': _f32(inputs['emb']),
        'xidx': idx_pack(inp),
        'xridx': idx_pack(tok_rev),
        'w_ih': w_ih, 'w_hh': w_hh, 'bias32': bias,
        'fcw': _bf(fcw_t), 'fcbR': fcbR,
        'ident': np.eye(128, dtype=np.float32),
        'ident_bf': _bf(np.eye(128, dtype=np.float32)),
        'transR': transR, 'iotaD': iotaD, 'iotaK': iotaK,
        'validT': _f32(validT.reshape(128, 16)),
        'ivT': _f32(ivT.reshape(128, 160)),
        'endSel': endSel, 'maskBT': maskBT,
        'hb_off': hb_off,
    }


# ----------------------------------------------------------------------------
# Device kernel
# ----------------------------------------------------------------------------

PHASE = int(os.environ.get('KPHASE', '9'))


def _build():
    nc = bacc.Bacc("TRN2", target_bir_lowering=False, debug=False,
                   num_devices=NC_)

    d_in = {}
    def din(name, shape, dt):
        d_in[name] = nc.dram_tensor(name, list(shape), dt, kind="ExternalInput").ap()
        return d_in[name]

    emb_d = din('emb', [V, E], F32)
    xidx_d = din('xidx', [128, NSLOT], I32)
    xridx_d = din('xridx', [128, NSLOT], I32)
    wih_d = din('w_ih', [128, 2 * 2 * 8 * 128], BF16)
    whh_d = din('w_hh', [128, 2 * 2 * 8 * 128], BF16)
    bias_d = din('bias32', [128, 16], F32)
    fcw_d = din('fcw', [128, 4 * K], BF16)
    fcb_d = din('fcbR', [128, K], F32)
    id_d = din('ident', [128, 128], F32)
    idbf_d = din('ident_bf', [128, 128], BF16)
    trans_d = din('transR', [128, K * K], F32)
    iotaD_d = din('iotaD', [128, K], F32)
    iotaK_d = din('iotaK', [128, K], F32)
    validT_d = din('validT', [128, 16], F32)
    ivT_d = din('ivT', [128, 160], F32)
    endSel_d = din('endSel', [128, T], F32)
    maskBT_d = din('maskBT', [128, T], F32)
    hboff_d = din('hb_off', [128, NSLOT], I32)

    out_d = nc.dram_tensor('out', [BL, T], F32, kind="ExternalOutput").ap()

    SIG = mybir.ActivationFunctionType.Sigmoid
    TANH = mybir.ActivationFunctionType.Tanh
    AL = mybir.AluOpType
    AX = mybir.AxisListType

    with tile.TileContext(nc) as tc:
        from contextlib import ExitStack
        ctx = ExitStack()
        cpool = ctx.enter_context(tc.tile_pool(name="consts", bufs=1))
        state = ctx.enter_context(tc.tile_pool(name="state", bufs=1))
        gather_p = ctx.enter_context(tc.tile_pool(name="gather", bufs=2))
        scratch = ctx.enter_context(tc.tile_pool(name="scratch", bufs=3))
        vit_p = ctx.enter_context(tc.tile_pool(name="vit", bufs=4))
        vbig = ctx.enter_context(tc.tile_pool(name="vbig", bufs=1))
        ps_tr = ctx.enter_context(tc.tile_pool(name="ps_tr", bufs=2, space="PSUM"))
        ps_mm = ctx.enter_context(tc.tile_pool(name="ps_mm", bufs=2, space="PSUM"))
        ps_g = ctx.enter_context(tc.tile_pool(name="ps_g", bufs=2, space="PSUM"))
        ps_fc = ctx.enter_context(tc.tile_pool(name="ps_fc", bufs=2, space="PSUM"))
        dram_p = ctx.enter_context(tc.tile_pool(name="dram", bufs=1, space="DRAM"))

        hb_dram_t = dram_p.tile([NBT, H], BF16)
        feats_dram_t = dram_p.tile([BL * T * K], F32)
        pre_dram_t = dram_p.tile([T * BL * K], F32)
        bp_dram_t = dram_p.tile([T * BL * K], F32)
        hb_dram = hb_dram_t[:]
        feats_dram = feats_dram_t[:]
        pre_dram = pre_dram_t[:]
        bp_dram = bp_dram_t[:]

        def load_const(dram, shape, dt, tag):
            t = cpool.tile(shape, dt, tag=tag)
            nc.sync.dma_start(t[:], dram)
            return t

        wih = load_const(wih_d[:], [128, 4096], BF16, tag='wih')
        whh = load_const(whh_d[:], [128, 4096], BF16, tag='whh')
        bias = load_const(bias_d[:], [128, 16], F32, tag='bias')
        fcw = load_const(fcw_d[:], [128, 4 * K], BF16, tag='fcw')
        fcbR = load_const(fcb_d[:], [128, K], F32, tag='fcbR')
        ident = load_const(id_d[:], [128, 128], F32, tag='ident')
        ident_bf = load_const(idbf_d[:], [128, 128], BF16, tag='ident_bf')
        transR = load_const(trans_d[:], [128, K * K], F32, tag='transR')
        iotaD = load_const(iotaD_d[:], [128, K], F32, tag='iotaD')
        iotaK = load_const(iotaK_d[:], [128, K], F32, tag='iotaK')
        validT = load_const(validT_d[:], [128, 16], F32, tag='validT')
        ivT = load_const(ivT_d[:], [128, 160], F32, tag='ivT')
        endSel = load_const(endSel_d[:], [128, T], F32, tag='endSel')
        maskBT = load_const(maskBT_d[:], [128, T], F32, tag='maskBT')
        xidx = load_const(xidx_d[:], [128, NSLOT], I32, tag='xidx')
        xridx = load_const(xridx_d[:], [128, NSLOT], I32, tag='xridx')
        hboff = load_const(hboff_d[:], [128, NSLOT], I32, tag='hboff')

        wih_r = wih[:].rearrange("p (d kc mc m) -> p d kc mc m", d=2, kc=2, mc=8)
        whh_r = whh[:].rearrange("p (d kc mc m) -> p d kc mc m", d=2, kc=2, mc=8)
        fcw_r = fcw[:].rearrange("p (c k) -> p c k", c=4)

        # ---- P1: embedding gather + transpose to x^T (E on partitions) ----
        gx_ctx = ExitStack()
        gxpool = gx_ctx.enter_context(tc.tile_pool(name="gxp", bufs=1))
        x_ctx = ExitStack()
        xpool = x_ctx.enter_context(tc.tile_pool(name="xp", bufs=1))
        x_bf = xpool.tile([128, 2 * 2 * NBT], BF16)   # [p, dir, ec, bt]
        xbf_r = x_bf[:].rearrange("p (d e n) -> p d e n", d=2, e=2)
        for d, idxt in ((0, xidx), (1, xridx)):
            for s_ in range(NSLOT):
                xs = gather_p.tile([128, E], F32, tag="xslot")
                nc.gpsimd.indirect_dma_start(
                    out=xs[:], out_offset=None, in_=emb_d,
                    in_offset=IndirectOffsetOnAxis(ap=idxt[:, s_:s_ + 1], axis=0),
                )
                for ec in range(2):
                    pt = ps_tr.tile([128, 128], F32, tag="ptr")
                    nc.tensor.transpose(out=pt[:], in_=xs[:, ec * 128:(ec + 1) * 128],
                                        identity=ident[:])
                    nc.vector.tensor_copy(
                        out=xbf_r[:, d, ec, s_ * 128:(s_ + 1) * 128], in_=pt[:])

        # ---- P2: gx = x @ Wih^T + bias (both dirs), bf16 store ----
        gx = gxpool.tile([128, 2 * 8 * NBT], BF16)     # [p, dir, mc, bt]
        gx_r = gx[:].rearrange("p (d mc n) -> p d mc n", d=2, mc=8)
        gx_rt = gx[:].rearrange("p (d mc b t) -> p d mc b t", d=2, mc=8, b=BL)
        NB = NBT // 512
        for d in range(2):
            for mc in range(8):
                for nb in range(NB):
                    pm = ps_mm.tile([128, 512], F32, tag="pmm")
                    for kc in range(2):
                        nc.tensor.matmul(
                            out=pm[:], lhsT=wih_r[:, d, kc, mc, :],
                            rhs=xbf_r[:, d, kc, nb * 512:(nb + 1) * 512],
                            start=(kc == 0), stop=(kc == 1))
                    nc.vector.tensor_scalar(
                        out=gx_r[:, d, mc, nb * 512:(nb + 1) * 512], in0=pm[:],
                        scalar1=bias[:, d * 8 + mc:d * 8 + mc + 1], scalar2=None,
                        op0=AL.add)

        x_ctx.close()

        # ---- P3: fused fwd+bwd LSTM scan ----
        hall = state.tile([128, 2 * 2 * (T + 1) * BL], BF16)  # [p, d, kc, t, b]
        hall_r = hall[:].rearrange("p (d kc t b) -> p d kc t b", d=2, kc=2, t=T + 1)
        cst = state.tile([128, 2 * 2 * BL], F32)              # [p, d, kc, b]
        cst_r = cst[:].rearrange("p (d kc b) -> p d kc b", d=2, kc=2)
        nc.vector.memset(hall_r[:, :, :, 0, :], 0.0)
        nc.vector.memset(cst[:], 0.0)

        for t in range(T):
            pg = ps_g.tile([128, 2 * 8 * BL], F32, tag="pg")   # [p, d, mc, b]
            pg_r = pg[:].rearrange("p (d mc b) -> p d mc b", d=2, mc=8)
            for d in range(2):
                for mc in range(8):
                    for kc in range(2):
                        nc.tensor.matmul(
                            out=pg_r[:, d, mc, :], lhsT=whh_r[:, d, kc, mc, :],
                            rhs=hall_r[:, d, kc, t, :],
                            start=(kc == 0), stop=(kc == 1))
            gsb = scratch.tile([128, 2 * 8 * BL], F32, tag="gsb")
            gsb_r = gsb[:].rearrange("p (d mc b) -> p d mc b", d=2, mc=8)
            nc.vector.tensor_tensor(out=gsb_r[:, :, :, :], in0=pg_r[:, :, :, :],
                                    in1=gx_rt[:, :, :, :, t], op=AL.add)
            sig = scratch.tile([128, 2 * 6 * BL], F32, tag="sig")
            sig_r = sig[:].rearrange("p (d c b) -> p d c b", d=2, c=6)
            nc.scalar.activation(out=sig_r[:, :, :, :], in_=gsb_r[:, :, 0:6, :], func=SIG)
            tg = scratch.tile([128, 2 * 2 * BL], F32, tag="tg")
            tg_r = tg[:].rearrange("p (d c b) -> p d c b", d=2, c=2)
            nc.scalar.activation(out=tg_r[:, :, :, :], in_=gsb_r[:, :, 6:8, :], func=TANH)

            t1 = scratch.tile([128, 2 * 2 * BL], F32, tag="t1")
            t1_r = t1[:].rearrange("p (d c b) -> p d c b", d=2, c=2)
            nc.vector.tensor_mul(out=t1_r[:, :, :, :], in0=sig_r[:, :, 0:2, :], in1=tg_r[:, :, :, :])
            cf = scratch.tile([128, 2 * 2 * BL], F32, tag="cf")
            cf_r = cf[:].rearrange("p (d c b) -> p d c b", d=2, c=2)
            nc.vector.tensor_mul(out=cf_r[:, :, :, :], in0=sig_r[:, :, 2:4, :], in1=cst_r[:, :, :, :])
            nc.vector.tensor_add(out=cst[:], in0=cf[:], in1=t1[:])
            tc_ = scratch.tile([128, 2 * 2 * BL], F32, tag="tc")
            tc_r = tc_[:].rearrange("p (d c b) -> p d c b", d=2, c=2)
            nc.scalar.activation(out=tc_r[:, :, :, :], in_=cst_r[:, :, :, :], func=TANH)
            nc.vector.tensor_mul(out=hall_r[:, :, :, t + 1, :], in0=sig_r[:, :, 4:6, :],
                                 in1=tc_r[:, :, :, :])

        gx_ctx.close()

        # ---- P4: hb re-reversal (DRAM bounce + indirect gather + transpose),
        #          then fc emissions ----
        # transpose hb (hid-on-partitions) -> scan-row tiles [bt, hid], dump to DRAM
        for s_ in range(NSLOT):
            hbs = gather_p.tile([128, H], BF16, tag="hbs")
            for ec in range(2):
                pt = ps_tr.tile([128, 128], BF16, tag="ptr")
                nc.tensor.transpose(
                    out=pt[:],
                    in_=hall_r[:, 1, ec, 1 + s_ * 16:1 + (s_ + 1) * 16, :],
                    identity=ident_bf[:])
                nc.vector.tensor_copy(out=hbs[:, ec * 128:(ec + 1) * 128], in_=pt[:])
            nc.sync.dma_start(out=hb_dram[s_ * 128:(s_ + 1) * 128, :], in_=hbs[:])
        hbT = state.tile([128, 2 * NBT], BF16)   # [p(hid), kc, bt]
        hbT_r = hbT[:].rearrange("p (kc n) -> p kc n", kc=2)
        for s_ in range(NSLOT):
            hs = gather_p.tile([128, H], BF16, tag="hslot")
            nc.gpsimd.indirect_dma_start(
                out=hs[:], out_offset=None, in_=hb_dram,
                in_offset=IndirectOffsetOnAxis(ap=hboff[:, s_:s_ + 1], axis=0))
            for ec in range(2):
                pt = ps_tr.tile([128, 128], BF16, tag="ptr")
                nc.tensor.transpose(out=pt[:], in_=hs[:, ec * 128:(ec + 1) * 128],
                                    identity=ident_bf[:])
                nc.vector.tensor_copy(out=hbT_r[:, ec, s_ * 128:(s_ + 1) * 128], in_=pt[:])

        feats_sb = state.tile([128, 16 * K], F32)   # [p, mt, k], bt = mt*128+p
        feats_r = feats_sb[:].rearrange("p (m k) -> p m k", m=16)
        for mt in range(16):
            b_, th = mt // 2, mt % 2
            pf = ps_fc.tile([128, K], F32, tag="pfc")
            for c4 in range(4):
                if c4 < 2:
                    lhs = hall_r[:, 0, c4, 1 + th * 128:1 + (th + 1) * 128, b_]
                else:
                    lhs = hbT_r[:, c4 - 2, mt * 128:(mt + 1) * 128]
                nc.tensor.matmul(out=pf[:], lhsT=lhs, rhs=fcw_r[:, c4, :],
                                 start=(c4 == 0), stop=(c4 == 3))
            nc.vector.tensor_tensor(out=feats_r[:, mt, :], in0=pf[:],
                                    in1=fcbR[:, :], op=AL.add)

        # relayout feats -> [b partitions, (t, k)] and add fc_b
        nc.sync.dma_start(
            out=feats_dram.rearrange("(b th p k) -> p b th k", b=BL, th=2, p=128),
            in_=feats_r[:, :, :].rearrange("p (b th) k -> p b th k", b=BL))
        feats8 = state.tile([128, T * K], F32)
        nc.sync.dma_start(out=feats8[0:BL, :],
                          in_=feats_dram.rearrange("(b tk) -> b tk", b=BL))
        f8_r = feats8[:].rearrange("p (t k) -> p t k", t=T)

        # ---- P5: Viterbi forward scan (VE-serial, batch on partitions) ----
        preH = state.tile([128, T * K], F32)
        preH_r = preH[:].rearrange("p (t k) -> p t k", t=T)
        nc.vector.tensor_copy(out=preH_r[0:BL, 0, :], in_=f8_r[0:BL, 0, :])
        for t in range(1, T):
            s1 = vit_p.tile([128, K * K], F32, tag="s1")
            nc.vector.tensor_tensor(
                out=s1[0:BL, :].rearrange("p (c q) -> p c q", c=K),
                in0=preH_r[0:BL, t - 1, :].unsqueeze(1)
                    .broadcast_to((BL, K, K)),
                in1=transR[0:BL, :].rearrange("p (c q) -> p c q", c=K), op=AL.add)
            m1 = vit_p.tile([128, K], F32, tag="m1")
            nc.vector.tensor_reduce(
                out=m1[0:BL, :], in_=s1[0:BL, :].rearrange("p (c q) -> p c q", c=K),
                axis=AX.X, op=AL.max)
            nc.vector.tensor_tensor(out=preH_r[0:BL, t, :], in0=m1[0:BL, :],
                                    in1=f8_r[0:BL, t, :], op=AL.add)

        # ---- P6: batched backpointer extraction ----
        nc.sync.dma_start(
            out=pre_dram.rearrange("(t b k) -> b t k", t=T, b=BL),
            in_=preH_r[0:BL, :, :])
        preT = vbig.tile([128, 2 * BL * K], F32, tag="preT")
        preT_r = preT[:].rearrange("p (h b k) -> p h b k", h=2, b=BL)
        nc.sync.dma_start(
            out=preT_r[:, :, :, :],
            in_=pre_dram.rearrange("(h p b k) -> p h b k", h=2, p=128, b=BL))

        HB = 2 * BL
        preT_hb = preT[:].rearrange("p (hb k) -> p hb k", k=K)
        sX = vbig.tile([128, 2 * BL * K * K], F32, tag="sX")
        sX_r = sX[:].rearrange("p (hb c q) -> p hb c q", hb=HB, c=K)
        nc.vector.tensor_tensor(
            out=sX_r[:, :, :, :],
            in0=preT_hb.unsqueeze(2).broadcast_to((128, HB, K, K)),
            in1=transR[:, :].rearrange("p (c q) -> p c q", c=K).unsqueeze(1)
                .broadcast_to((128, HB, K, K)),
            op=AL.add)
        mX = vbig.tile([128, 2 * BL * K], F32, tag="mX")
        mX_r = mX[:].rearrange("p (hb c) -> p hb c", hb=HB)
        nc.vector.tensor_reduce(out=mX_r[:, :, :], in_=sX_r[:, :, :, :],
                                axis=AX.X, op=AL.max)
        eq = vbig.tile([128, 2 * BL * K * K], F32, tag="eq")
        eq_r = eq[:].rearrange("p (hb c q) -> p hb c q", hb=HB, c=K)
        nc.vector.tensor_tensor(
            out=eq_r[:, :, :, :], in0=sX_r[:, :, :, :],
            in1=mX_r[:, :, :].unsqueeze(3).broadcast_to((128, HB, K, K)),
            op=AL.is_equal)
        nc.vector.tensor_tensor(
            out=eq_r[:, :, :, :], in0=eq_r[:, :, :, :],
            in1=iotaD[:, :].unsqueeze(1).unsqueeze(1).broadcast_to((128, HB, K, K)),
            op=AL.mult)
        bq = vbig.tile([128, 2 * BL * K], F32, tag="bq")
        bq_r = bq[:].rearrange("p (hb c) -> p hb c", hb=HB)
        nc.vector.tensor_reduce(out=bq_r[:, :, :], in_=eq_r[:, :, :, :],
                                axis=AX.X, op=AL.max)
        # bp = 9 - bq ; then pad override: bp*valid + iota_cur*(1-valid)
        nc.vector.tensor_scalar(out=bq[:], in0=bq[:], scalar1=-1.0, scalar2=9.0,
                                op0=AL.mult, op1=AL.add)
        nc.vector.tensor_tensor(
            out=bq_r[:, :, :], in0=bq_r[:, :, :],
            in1=validT[:, :].unsqueeze(2).broadcast_to((128, HB, K)),
            op=AL.mult)
        nc.vector.tensor_tensor(
            out=bq_r[:, :, :], in0=bq_r[:, :, :],
            in1=ivT[:, :].rearrange("p (hb k) -> p hb k", k=K),
            op=AL.add)
        bq4 = bq[:].rearrange("p (h b k) -> p h b k", h=2, b=BL)
        nc.sync.dma_start(
            out=bp_dram.rearrange("(h p b k) -> p h b k", h=2, p=128, b=BL),
            in_=bq4)
        bp8 = state.tile([128, T * K], F32)
        nc.sync.dma_start(
            out=bp8[0:BL, :].rearrange("p (t k) -> p t k", t=T),
            in_=bp_dram.rearrange("(t b k) -> b t k", t=T, b=BL))
        bp8_r = bp8[:].rearrange("p (t k) -> p t k", t=T)

        # ---- P7: end-tag + backtrace ----
        best = state.tile([128, T], F32)
        pesel = vbig.tile([128, T * K], F32, tag="pesel")
        nc.vector.tensor_tensor(
            out=pesel[0:BL, :].rearrange("p (t k) -> p t k", t=T),
            in0=preH_r[0:BL, :, :],
            in1=endSel[0:BL, :].unsqueeze(2).broadcast_to((BL, T, K)),
            op=AL.mult)
        pe = vbig.tile([128, K], F32, tag="pe")
        nc.vector.tensor_reduce(
            out=pe[0:BL, :],
            in_=pesel[0:BL, :].rearrange("p (t k) -> p k t", t=T),
            axis=AX.X, op=AL.max)
        mvE = vit_p.tile([128, 1], F32, tag="mvE")
        nc.vector.tensor_reduce(out=mvE[0:BL, :], in_=pe[0:BL, :], axis=AX.X, op=AL.max)
        eqE = vit_p.tile([128, K], F32, tag="eqE")
        nc.vector.tensor_tensor(out=eqE[0:BL, :], in0=pe[0:BL, :],
                                in1=mvE[0:BL, :].broadcast_to((BL, K)), op=AL.is_equal)
        nc.vector.tensor_mul(out=eqE[0:BL, :], in0=eqE[0:BL, :], in1=iotaD[0:BL, :])
        eT = vit_p.tile([128, 1], F32, tag="eT")
        nc.vector.tensor_reduce(out=eT[0:BL, :], in_=eqE[0:BL, :], axis=AX.X, op=AL.max)
        nc.vector.tensor_scalar(out=best[0:BL, T - 1:T], in0=eT[0:BL, :],
                                scalar1=-1.0, scalar2=9.0, op0=AL.mult, op1=AL.add)

        for t in range(T - 2, -1, -1):
            oh = vit_p.tile([128, K], F32, tag="oh")
            nc.vector.tensor_tensor(
                out=oh[0:BL, :], in0=iotaK[0:BL, :],
                in1=best[0:BL, t + 1:t + 2].broadcast_to((BL, K)), op=AL.is_equal)
            dmy = vit_p.tile([128, K], F32, tag="dmy")
            nc.vector.tensor_mul(out=dmy[0:BL, :], in0=bp8_r[0:BL, t, :], in1=oh[0:BL, :])
            nc.vector.tensor_reduce(out=best[0:BL, t:t + 1], in_=dmy[0:BL, :],
                                    axis=AX.X, op=AL.max)

        bestM = state.tile([128, T], F32)
        nc.vector.tensor_mul(out=bestM[0:BL, :], in0=best[0:BL, :], in1=maskBT[0:BL, :])
        nc.sync.dma_start(out=out_d, in_=bestM[0:BL, :])
        ctx.close()

    nc.compile()
    return nc


_NC_CACHE = None


def _get_nc():
    global _NC_CACHE
    if _NC_CACHE is None:
        _NC_CACHE = _build()
    return _NC_CACHE


TRACE = False
LAST_EXEC_NS = None


def kernel(**inputs) -> np.ndarray:
    global LAST_EXEC_NS
    nc = _get_nc()
    in_maps = [_prep_core(inputs, c) for c in range(NC_)]
    res = run_bass_kernel_spmd(nc, in_maps, list(range(NC_)), trace=TRACE)
    LAST_EXEC_NS = res.exec_time_ns
    out = np.concatenate([res.results[c]['out'] for c in range(NC_)], axis=0)
    return out.astype(np.float32)


if __name__ == '__main__':
    _build()
    print("build ok")



# revision 8
# speedup vs baseline: 1.2119x; 1.2119x over previous
"""BiLSTM-CRF (Viterbi decode) Trainium2 Bass kernel, 8-core data-parallel.

Full inputs in, full outputs out. Batch (64) is sharded 8 ways; each core runs:
  bf16 embedding gather -> input matmuls (gx = x @ Wih^T + b) -> 256-step fused
  fwd+bwd LSTM recurrence -> fc emissions -> Viterbi scan -> batched
  backpointer extraction -> backtrace.

Layout: gate/hidden dims live on SBUF partitions, (t, b) t-major on the free
dim. The LSTM scan folds the gx add into PSUM via an identity matmul and keeps
a minimal serial chain: sig(i,f) -> tanh(g) -> t1 -> c-add -> tanh(c) -> h.
"""

import os
import sys
import types

for _p in ('/opt/trn_rl_repo', '/root/.axon_site'):
    if _p not in sys.path:
        sys.path.insert(0, _p)

import numpy as np
import ml_dtypes

# ---- NTFF profile hook (lets run_bass_kernel_spmd(trace=True) return timings
# under axon; harmless if already registered or unavailable) ----
def _install_ntff_hook():
    try:
        import antenv
        if 'antenv.axon_hooks' in sys.modules:
            return
        from trn_agent_boot.trn_boot import _ntff_profile_via_ctypes
        m = types.ModuleType('antenv.axon_hooks')
        m._hook = _ntff_profile_via_ctypes('/opt/axon/libaxon_pjrt.so')
        m.get_axon_ntff_profile_hook = lambda: m._hook
        m.set_axon_ntff_profile_hook = lambda h: setattr(m, '_hook', h)
        sys.modules['antenv.axon_hooks'] = m
        antenv.axon_hooks = m
    except Exception:
        pass


_install_ntff_hook()

import concourse.bass as bass
import concourse.tile as tile
from concourse import bacc, mybir
from concourse.bass import IndirectOffsetOnAxis
from concourse.bass_utils import run_bass_kernel_spmd

F32 = mybir.dt.float32
BF16 = mybir.dt.bfloat16
I32 = mybir.dt.int32

# Problem dims (hardcoded per contract)
V, E, HS, T, B = 30000, 256, 512, 256, 64
H = HS // 2          # 256 per-direction hidden
G = 4 * H            # 1024 gate rows per direction
K = 10               # tags
NC_ = 8              # cores
BL = B // NC_        # 8 sequences per core
NBT = BL * T         # 2048 (t,b) columns per core, t-major: j = t*8 + b
NSLOT = NBT // 128   # 16 gather slots

# Gate reorder: torch rows [i, f, g, o] -> device order [i, f, o, g]
_PERM = np.concatenate([
    np.arange(0, 2 * H),          # i, f
    np.arange(3 * H, 4 * H),      # o
    np.arange(2 * H, 3 * H),      # g
])

# gx column layout within a timestep: 128 cols = (tile-group, d, mc-within, b)
# groups: IF (d,mc0-3)->pos 0-7, G (d,mc6-7)->pos 8-11... we use
# col = POS(d, mc)*8 + b with POS ordering: IF: d*4 + mc (mc<4) in [0,8);
# O: 8 + d*2 + (mc-4) in [8,12)? NO -- order chosen: IF block cols 0:64,
# G block cols 64:96, O block cols 96:128.
def _gx_pos(d, mc):
    if mc < 4:                    # i, f
        return d * 4 + mc         # 0..7
    if mc >= 6:                   # g
        return 8 + d * 2 + (mc - 6)   # 8..11
    return 12 + d * 2 + (mc - 4)      # o: 12..15


def _bf(x):
    return np.ascontiguousarray(np.asarray(x, np.float32).astype(ml_dtypes.bfloat16))


def _f32(x):
    return np.ascontiguousarray(np.asarray(x, np.float32))


def _pack_w(wih, whh, bih, bhh):
    """Per direction: returns (w_ih[128, 2*8*128], w_hh[...], bias[128, 8]) in
    lhsT tile layout w[p, kc, mc, m] = W[perm[mc*128+m], kc*128+p]."""
    out = []
    for W in (wih, whh):
        Wp = np.asarray(W, np.float32)[_PERM]          # [G, Kdim]
        Kd = Wp.shape[1]
        t = Wp.reshape(8, 128, Kd // 128, 128)          # [mc, m, kc, p]
        t = np.transpose(t, (3, 2, 0, 1))               # [p, kc, mc, m]
        out.append(t.reshape(128, -1))
    b = (np.asarray(bih, np.float32) + np.asarray(bhh, np.float32))[_PERM]
    b = b.reshape(8, 128).T                             # [p, mc]
    return out[0], out[1], b


_EMB_BF_CACHE = None


def _prep_core(inputs, core):
    """Host-side prep of all per-core device inputs."""
    global _EMB_BF_CACHE
    s = slice(core * BL, (core + 1) * BL)
    inp = np.asarray(inputs['inp'])[s]        # [8, 256] int
    n = np.asarray(inputs['n'])[s].astype(np.int64)

    t_idx = np.arange(T)
    mask = t_idx[None, :] < n[:, None]
    rev = np.where(mask, n[:, None] - 1 - t_idx[None, :], t_idx[None, :])
    tok_rev = np.take_along_axis(inp, rev, axis=1)

    def idx_pack(tok):  # [8,256] -> [128, 16] slot layout, t-major j = t*8+b
        flat = np.asarray(tok, np.int64).T.reshape(-1)
        return flat.reshape(NSLOT, 128).T.astype(np.int32).copy()

    wf = _pack_w(inputs['W_ih_f'], inputs['W_hh_f'], inputs['b_ih_f'], inputs['b_hh_f'])
    wb = _pack_w(inputs['W_ih_b'], inputs['W_hh_b'], inputs['b_ih_b'], inputs['b_hh_b'])
    w_ih = _bf(np.concatenate([wf[0], wb[0]], axis=1))   # [128, 2*2048]
    w_hh = _bf(np.concatenate([wf[1], wb[1]], axis=1))
    bias = _f32(np.concatenate([wf[2], wb[2]], axis=1))  # [128, 16] (d, mc)

    fcw = np.asarray(inputs['fc_w'], np.float32)         # [10, 512]
    fcw_t = fcw.T.reshape(4, 128, K).transpose(1, 0, 2).reshape(128, 4 * K)
    fcbR = np.tile(np.asarray(inputs['fc_b'], np.float32)[None, :], (128, 1))

    trans = np.asarray(inputs['transition'], np.float32)[:K, :K]  # [prev, cur]
    transR = np.tile(trans.T.reshape(1, K * K), (128, 1)).copy()  # [p, cur*10+prev]

    iotaD = np.tile((9.0 - np.arange(K, dtype=np.float32))[None, :], (128, 1))
    iotaK = np.tile(np.arange(K, dtype=np.float32)[None, :], (128, 1))

    # validT[p, h*8+b] = ((h*128+p)+1 < n_b); ivT[p, (h,b,k)] = k*(1-valid)
    tt = (np.arange(256).reshape(2, 128).T)[:, :, None]            # [p, h, 1]
    validT = (tt + 1 < n[None, None, :]).astype(np.float32)        # [p, h, b]
    ivT = (1.0 - validT)[:, :, :, None] * np.arange(K, dtype=np.float32)[None, None, None, :]

    endSel = np.zeros((128, T), np.float32)
    endSel[np.arange(BL), (n - 1)] = 1.0
    maskBT = np.zeros((128, T), np.float32)
    maskBT[:BL] = mask.astype(np.float32)

    # hb re-reversal gather rows: out col j'=(t,b) <- hb_dram row rev[b,t]*8 + b
    hb_rows = (rev.T * 8 + np.arange(BL)[None, :]).reshape(-1)     # t-major
    hb_off = hb_rows.reshape(NSLOT, 128).T.astype(np.int32).copy()

    if _EMB_BF_CACHE is None:
        _EMB_BF_CACHE = _bf(inputs['emb'])

    return {
        'emb': _EMB_BF_CACHE,
        'xidx': idx_pack(inp),
        'xridx': idx_pack(tok_rev),
        'w_ih': w_ih, 'w_hh': w_hh, 'bias32': bias,
        'fcw': _bf(fcw_t), 'fcbR': fcbR,
        'ident_bf': _bf(np.eye(128, dtype=np.float32)),
        'transR': transR, 'iotaD': iotaD, 'iotaK': iotaK,
        'validT': _f32(validT.reshape(128, 16)),
        'ivT': _f32(ivT.reshape(128, 160)),
        'endSel': endSel, 'maskBT': maskBT,
        'hb_off': hb_off,
    }


# ----------------------------------------------------------------------------
# Device kernel
# ----------------------------------------------------------------------------

def _build():
    nc = bacc.Bacc("TRN2", target_bir_lowering=False, debug=False,
                   num_devices=NC_)

    d_in = {}
    def din(name, shape, dt):
        d_in[name] = nc.dram_tensor(name, list(shape), dt, kind="ExternalInput").ap()
        return d_in[name]

    emb_d = din('emb', [V, E], BF16)
    xidx_d = din('xidx', [128, NSLOT], I32)
    xridx_d = din('xridx', [128, NSLOT], I32)
    wih_d = din('w_ih', [128, 2 * 2 * 8 * 128], BF16)
    whh_d = din('w_hh', [128, 2 * 2 * 8 * 128], BF16)
    bias_d = din('bias32', [128, 16], F32)
    fcw_d = din('fcw', [128, 4 * K], BF16)
    fcb_d = din('fcbR', [128, K], F32)
    idbf_d = din('ident_bf', [128, 128], BF16)
    trans_d = din('transR', [128, K * K], F32)
    iotaD_d = din('iotaD', [128, K], F32)
    iotaK_d = din('iotaK', [128, K], F32)
    validT_d = din('validT', [128, 16], F32)
    ivT_d = din('ivT', [128, 160], F32)
    endSel_d = din('endSel', [128, T], F32)
    maskBT_d = din('maskBT', [128, T], F32)
    hboff_d = din('hb_off', [128, NSLOT], I32)

    out_d = nc.dram_tensor('out', [BL, T], F32, kind="ExternalOutput").ap()

    SIG = mybir.ActivationFunctionType.Sigmoid
    TANH = mybir.ActivationFunctionType.Tanh
    AL = mybir.AluOpType
    AX = mybir.AxisListType

    with tile.TileContext(nc) as tc:
        from contextlib import ExitStack
        ctx = ExitStack()
        cpool = ctx.enter_context(tc.tile_pool(name="consts", bufs=1))
        state = ctx.enter_context(tc.tile_pool(name="state", bufs=1))
        gather_p = ctx.enter_context(tc.tile_pool(name="gather", bufs=2))
        scratch = ctx.enter_context(tc.tile_pool(name="scratch", bufs=3))
        vit_p = ctx.enter_context(tc.tile_pool(name="vit", bufs=4))
        vbig = ctx.enter_context(tc.tile_pool(name="vbig", bufs=1))
        dram_p = ctx.enter_context(tc.tile_pool(name="dram", bufs=1, space="DRAM"))

        hb_dram_t = dram_p.tile([NBT, H], BF16)
        feats_dram_t = dram_p.tile([BL * T * K], F32)
        pre_dram_t = dram_p.tile([T * BL * K], F32)
        bp_dram_t = dram_p.tile([T * BL * K], F32)
        hb_dram = hb_dram_t[:]
        feats_dram = feats_dram_t[:]
        pre_dram = pre_dram_t[:]
        bp_dram = bp_dram_t[:]

        def load_const(dram, shape, dt, tag):
            t = cpool.tile(shape, dt, tag=tag)
            nc.sync.dma_start(t[:], dram)
            return t

        wih = load_const(wih_d[:], [128, 4096], BF16, tag='wih')
        whh = load_const(whh_d[:], [128, 4096], BF16, tag='whh')
        bias = load_const(bias_d[:], [128, 16], F32, tag='bias')
        fcw = load_const(fcw_d[:], [128, 4 * K], BF16, tag='fcw')
        fcbR = load_const(fcb_d[:], [128, K], F32, tag='fcbR')
        ident_bf = load_const(idbf_d[:], [128, 128], BF16, tag='ident_bf')
        transR = load_const(trans_d[:], [128, K * K], F32, tag='transR')
        iotaD = load_const(iotaD_d[:], [128, K], F32, tag='iotaD')
        iotaK = load_const(iotaK_d[:], [128, K], F32, tag='iotaK')
        validT = load_const(validT_d[:], [128, 16], F32, tag='validT')
        ivT = load_const(ivT_d[:], [128, 160], F32, tag='ivT')
        endSel = load_const(endSel_d[:], [128, T], F32, tag='endSel')
        maskBT = load_const(maskBT_d[:], [128, T], F32, tag='maskBT')
        xidx = load_const(xidx_d[:], [128, NSLOT], I32, tag='xidx')
        xridx = load_const(xridx_d[:], [128, NSLOT], I32, tag='xridx')
        hboff = load_const(hboff_d[:], [128, NSLOT], I32, tag='hboff')

        wih_r = wih[:].rearrange("p (d kc mc m) -> p d kc mc m", d=2, kc=2, mc=8)
        whh_r = whh[:].rearrange("p (d kc mc m) -> p d kc mc m", d=2, kc=2, mc=8)
        fcw_r = fcw[:].rearrange("p (c k) -> p c k", c=4)

        # ---- P1: embedding gather (bf16) + transpose to x^T (E on parts) ----
        gx_ctx = ExitStack()
        gxpool = gx_ctx.enter_context(tc.tile_pool(name="gxp", bufs=1))
        x_ctx = ExitStack()
        xpool = x_ctx.enter_context(tc.tile_pool(name="xp", bufs=1))
        ps_tr = x_ctx.enter_context(tc.tile_pool(name="ps_tr", bufs=2, space="PSUM"))
        ps_mm = x_ctx.enter_context(tc.tile_pool(name="ps_mm", bufs=2, space="PSUM"))
        x_bf = xpool.tile([128, 2 * 2 * NBT], BF16)   # [p, dir, ec, bt']
        xbf_r = x_bf[:].rearrange("p (d e n) -> p d e n", d=2, e=2)
        for d, idxt in ((0, xidx), (1, xridx)):
            for s_ in range(NSLOT):
                xs = gather_p.tile([128, E], BF16, tag="xslot")
                nc.gpsimd.indirect_dma_start(
                    out=xs[:], out_offset=None, in_=emb_d,
                    in_offset=IndirectOffsetOnAxis(ap=idxt[:, s_:s_ + 1], axis=0),
                )
                for ec in range(2):
                    pt = ps_tr.tile([128, 128], BF16, tag="ptr")
                    nc.tensor.transpose(out=pt[:], in_=xs[:, ec * 128:(ec + 1) * 128],
                                        identity=ident_bf[:])
                    nc.vector.tensor_copy(
                        out=xbf_r[:, d, ec, s_ * 128:(s_ + 1) * 128], in_=pt[:])

        # ---- P2: gx = x @ Wih^T + bias (both dirs), bf16, scan layout ----
        # gx cols: t*128 + pos(d,mc)*8 + b  (IF block 0:64, G 64:96, O 96:128)
        gx = gxpool.tile([128, T * 128], BF16)
        gx_v = gx[:].rearrange("p (t c) -> p t c", t=T)
        gx_tb = gx[:].rearrange("p (t c b) -> p t c b", t=T, c=16)
        NB = NBT // 512
        for nb in range(NB):
            t0 = nb * 64
            for d in range(2):
                for mc in range(8):
                    pm = ps_mm.tile([128, 512], F32, tag="pmm")
                    for kc in range(2):
                        nc.tensor.matmul(
                            out=pm[:], lhsT=wih_r[:, d, kc, mc, :],
                            rhs=xbf_r[:, d, kc, nb * 512:(nb + 1) * 512],
                            start=(kc == 0), stop=(kc == 1))
                    pos = _gx_pos(d, mc)
                    nc.vector.tensor_scalar(
                        out=gx_tb[:, t0:t0 + 64, pos, :],
                        in0=pm[:].rearrange("p (t b) -> p t b", t=64),
                        scalar1=bias[:, d * 8 + mc:d * 8 + mc + 1], scalar2=None,
                        op0=AL.add)

        x_ctx.close()

        # ---- P3: fused fwd+bwd LSTM scan ----
        scan_ctx = ExitStack()
        ps_if = scan_ctx.enter_context(tc.tile_pool(name="ps_if", bufs=2, space="PSUM"))
        ps_g = scan_ctx.enter_context(tc.tile_pool(name="ps_g", bufs=2, space="PSUM"))
        ps_o = scan_ctx.enter_context(tc.tile_pool(name="ps_o", bufs=2, space="PSUM"))
        hall = state.tile([128, 2 * 2 * (T + 1) * BL], BF16)  # [p, d, kc, t, b]
        hall_r = hall[:].rearrange("p (d kc t b) -> p d kc t b", d=2, kc=2, t=T + 1)
        cst = state.tile([128, 2 * 2 * BL], F32)              # [p, d, kc, b]
        cst_r = cst[:].rearrange("p (d kc b) -> p d kc b", d=2, kc=2)
        nc.vector.memset(hall_r[:, :, :, 0, :], 0.0)
        nc.vector.memset(cst[:], 0.0)

        # weight tile order per group: IF: (d, mc 0..3), G: (d, mc 6..7), O: (d, mc 4..5)
        for t in range(T):
            pif = ps_if.tile([128, 64], F32, tag="pif")    # (d, mc0-3, b)
            pg = ps_g.tile([128, 32], F32, tag="pg")       # (d, g0-1, b)
            po = ps_o.tile([128, 32], F32, tag="po")       # (d, o0-1, b)
            pif_r = pif[:].rearrange("p (d m b) -> p d m b", d=2, m=4)
            pg_r = pg[:].rearrange("p (d m b) -> p d m b", d=2, m=2)
            po_r = po[:].rearrange("p (d m b) -> p d m b", d=2, m=2)
            # IF group
            nc.tensor.matmul(out=pif[:], lhsT=ident_bf[:], rhs=gx_v[:, t, 0:64],
                             start=True, stop=False)
            for d in range(2):
                for mi in range(4):
                    for kc in range(2):
                        nc.tensor.matmul(
                            out=pif_r[:, d, mi, :], lhsT=whh_r[:, d, kc, mi, :],
                            rhs=hall_r[:, d, kc, t, :],
                            start=False, stop=(kc == 1))
            # G group
            nc.tensor.matmul(out=pg[:], lhsT=ident_bf[:], rhs=gx_v[:, t, 64:96],
                             start=True, stop=False)
            for d in range(2):
                for mi in range(2):
                    for kc in range(2):
                        nc.tensor.matmul(
                            out=pg_r[:, d, mi, :], lhsT=whh_r[:, d, kc, 6 + mi, :],
                            rhs=hall_r[:, d, kc, t, :],
                            start=False, stop=(kc == 1))
            # O group
            nc.tensor.matmul(out=po[:], lhsT=ident_bf[:], rhs=gx_v[:, t, 96:128],
                             start=True, stop=False)
            for d in range(2):
                for mi in range(2):
                    for kc in range(2):
                        nc.tensor.matmul(
                            out=po_r[:, d, mi, :], lhsT=whh_r[:, d, kc, 4 + mi, :],
                            rhs=hall_r[:, d, kc, t, :],
                            start=False, stop=(kc == 1))

            sigIF = scratch.tile([128, 64], F32, tag="sigIF")
            sif_r = sigIF[:].rearrange("p (d m b) -> p d m b", d=2, m=4)
            nc.scalar.activation(out=sigIF[:], in_=pif[:], func=SIG)
            tg = scratch.tile([128, 32], F32, tag="tg")
            nc.scalar.activation(out=tg[:], in_=pg[:], func=TANH)
            so = scratch.tile([128, 32], F32, tag="so")
            nc.scalar.activation(out=so[:], in_=po[:], func=SIG)

            cf = scratch.tile([128, 32], F32, tag="cf")
            cf_r = cf[:].rearrange("p (d c b) -> p d c b", d=2, c=2)
            nc.vector.tensor_mul(out=cf_r[:, :, :, :], in0=sif_r[:, :, 2:4, :],
                                 in1=cst_r[:, :, :, :])
            t1 = scratch.tile([128, 32], F32, tag="t1")
            t1_r = t1[:].rearrange("p (d c b) -> p d c b", d=2, c=2)
            nc.vector.tensor_mul(out=t1_r[:, :, :, :], in0=sif_r[:, :, 0:2, :],
                                 in1=tg[:].rearrange("p (d c b) -> p d c b", d=2, c=2))
            nc.vector.tensor_add(out=cst[:], in0=cf[:], in1=t1[:])
            tc_ = scratch.tile([128, 32], F32, tag="tc")
            nc.scalar.activation(out=tc_[:], in_=cst[:], func=TANH)
            nc.vector.tensor_mul(
                out=hall_r[:, :, :, t + 1, :],
                in0=so[:].rearrange("p (d c b) -> p d c b", d=2, c=2),
                in1=tc_[:].rearrange("p (d c b) -> p d c b", d=2, c=2))

        gx_ctx.close()
        scan_ctx.close()

        # ---- P4: hb re-reversal (DRAM bounce + indirect gather + transpose),
        #          then fc emissions; all (t, b) t-major ----
        p4_ctx = ExitStack()
        ps_tr = p4_ctx.enter_context(tc.tile_pool(name="ps_tr2", bufs=2, space="PSUM"))
        ps_fc = p4_ctx.enter_context(tc.tile_pool(name="ps_fc", bufs=2, space="PSUM"))
        for s_ in range(NSLOT):
            hbs = gather_p.tile([128, H], BF16, tag="hbs")
            for ec in range(2):
                pt = ps_tr.tile([128, 128], BF16, tag="ptr")
                nc.tensor.transpose(
                    out=pt[:],
                    in_=hall_r[:, 1, ec, 1 + s_ * 16:1 + (s_ + 1) * 16, :],
                    identity=ident_bf[:])
                nc.vector.tensor_copy(out=hbs[:, ec * 128:(ec + 1) * 128], in_=pt[:])
            nc.sync.dma_start(out=hb_dram[s_ * 128:(s_ + 1) * 128, :], in_=hbs[:])
        hbT = state.tile([128, 2 * NBT], BF16)   # [p(hid), kc, bt']
        hbT_r = hbT[:].rearrange("p (kc n) -> p kc n", kc=2)
        for s_ in range(NSLOT):
            hs = gather_p.tile([128, H], BF16, tag="hslot")
            nc.gpsimd.indirect_dma_start(
                out=hs[:], out_offset=None, in_=hb_dram,
                in_offset=IndirectOffsetOnAxis(ap=hboff[:, s_:s_ + 1], axis=0))
            for ec in range(2):
                pt = ps_tr.tile([128, 128], BF16, tag="ptr")
                nc.tensor.transpose(out=pt[:], in_=hs[:, ec * 128:(ec + 1) * 128],
                                    identity=ident_bf[:])
                nc.vector.tensor_copy(out=hbT_r[:, ec, s_ * 128:(s_ + 1) * 128], in_=pt[:])

        feats_sb = state.tile([128, 16 * K], F32)   # [p=(tw,b), mt, k]
        feats_r = feats_sb[:].rearrange("p (m k) -> p m k", m=16)
        for mt in range(16):
            pf = ps_fc.tile([128, K], F32, tag="pfc")
            for c4 in range(4):
                if c4 < 2:
                    lhs = hall_r[:, 0, c4, 1 + mt * 16:1 + (mt + 1) * 16, :]
                else:
                    lhs = hbT_r[:, c4 - 2, mt * 128:(mt + 1) * 128]
                nc.tensor.matmul(out=pf[:], lhsT=lhs, rhs=fcw_r[:, c4, :],
                                 start=(c4 == 0), stop=(c4 == 3))
            nc.vector.tensor_tensor(out=feats_r[:, mt, :], in0=pf[:],
                                    in1=fcbR[:, :], op=AL.add)
        p4_ctx.close()

        # relayout feats -> [b partitions, (t, k)]; p=(tw, b), t = mt*16+tw
        # bounce stored (tw, b, mt, k); load permutes to (b, mt, tw, k) = (b, t, k)
        nc.sync.dma_start(
            out=feats_dram.rearrange("(tw b mt k) -> (tw b) mt k", tw=16, b=BL, mt=16),
            in_=feats_r[:, :, :])
        feats8 = state.tile([128, T * K], F32)
        nc.sync.dma_start(
            out=feats8[0:BL, :].rearrange("p (mt tw k) -> p mt tw k", mt=16, tw=16),
            in_=feats_dram.rearrange("(tw b mt k) -> b mt tw k", tw=16, b=BL, mt=16))
        f8_r = feats8[:].rearrange("p (t k) -> p t k", t=T)

        # ---- P5: Viterbi forward scan (VE-serial, batch on partitions) ----
        preH = state.tile([128, T * K], F32)
        preH_r = preH[:].rearrange("p (t k) -> p t k", t=T)
        nc.vector.tensor_copy(out=preH_r[0:BL, 0, :], in_=f8_r[0:BL, 0, :])
        for t in range(1, T):
            s1 = vit_p.tile([128, K * K], F32, tag="s1")
            nc.vector.tensor_tensor(
                out=s1[0:BL, :].rearrange("p (c q) -> p c q", c=K),
                in0=preH_r[0:BL, t - 1, :].unsqueeze(1)
                    .broadcast_to((BL, K, K)),
                in1=transR[0:BL, :].rearrange("p (c q) -> p c q", c=K), op=AL.add)
            m1 = vit_p.tile([128, K], F32, tag="m1")
            nc.vector.tensor_reduce(
                out=m1[0:BL, :], in_=s1[0:BL, :].rearrange("p (c q) -> p c q", c=K),
                axis=AX.X, op=AL.max)
            nc.vector.tensor_tensor(out=preH_r[0:BL, t, :], in0=m1[0:BL, :],
                                    in1=f8_r[0:BL, t, :], op=AL.add)

        # ---- P6: batched backpointer extraction ----
        nc.sync.dma_start(
            out=pre_dram.rearrange("(t b k) -> b t k", t=T, b=BL),
            in_=preH_r[0:BL, :, :])
        preT = vbig.tile([128, 2 * BL * K], F32, tag="preT")
        preT_r = preT[:].rearrange("p (h b k) -> p h b k", h=2, b=BL)
        nc.sync.dma_start(
            out=preT_r[:, :, :, :],
            in_=pre_dram.rearrange("(h p b k) -> p h b k", h=2, p=128, b=BL))

        HB = 2 * BL
        preT_hb = preT[:].rearrange("p (hb k) -> p hb k", k=K)
        sX = vbig.tile([128, 2 * BL * K * K], F32, tag="sX")
        sX_r = sX[:].rearrange("p (hb c q) -> p hb c q", hb=HB, c=K)
        nc.vector.tensor_tensor(
            out=sX_r[:, :, :, :],
            in0=preT_hb.unsqueeze(2).broadcast_to((128, HB, K, K)),
            in1=transR[:, :].rearrange("p (c q) -> p c q", c=K).unsqueeze(1)
                .broadcast_to((128, HB, K, K)),
            op=AL.add)
        mX = vbig.tile([128, 2 * BL * K], F32, tag="mX")
        mX_r = mX[:].rearrange("p (hb c) -> p hb c", hb=HB)
        nc.vector.tensor_reduce(out=mX_r[:, :, :], in_=sX_r[:, :, :, :],
                                axis=AX.X, op=AL.max)
        eq = vbig.tile([128, 2 * BL * K * K], F32, tag="eq")
        eq_r = eq[:].rearrange("p (hb c q) -> p hb c q", hb=HB, c=K)
        nc.vector.tensor_tensor(
            out=eq_r[:, :, :, :], in0=sX_r[:, :, :, :],
            in1=mX_r[:, :, :].unsqueeze(3).broadcast_to((128, HB, K, K)),
            op=AL.is_equal)
        nc.vector.tensor_tensor(
            out=eq_r[:, :, :, :], in0=eq_r[:, :, :, :],
            in1=iotaD[:, :].unsqueeze(1).unsqueeze(1).broadcast_to((128, HB, K, K)),
            op=AL.mult)
        bq = vbig.tile([128, 2 * BL * K], F32, tag="bq")
        bq_r = bq[:].rearrange("p (hb c) -> p hb c", hb=HB)
        nc.vector.tensor_reduce(out=bq_r[:, :, :], in_=eq_r[:, :, :, :],
                                axis=AX.X, op=AL.max)
        # bp = 9 - bq ; then pad override: bp*valid + iota_cur*(1-valid)
        nc.vector.tensor_scalar(out=bq[:], in0=bq[:], scalar1=-1.0, scalar2=9.0,
                                op0=AL.mult, op1=AL.add)
        nc.vector.tensor_tensor(
            out=bq_r[:, :, :], in0=bq_r[:, :, :],
            in1=validT[:, :].unsqueeze(2).broadcast_to((128, HB, K)),
            op=AL.mult)
        nc.vector.tensor_tensor(
            out=bq_r[:, :, :], in0=bq_r[:, :, :],
            in1=ivT[:, :].rearrange("p (hb k) -> p hb k", k=K),
            op=AL.add)
        bq4 = bq[:].rearrange("p (h b k) -> p h b k", h=2, b=BL)
        nc.sync.dma_start(
            out=bp_dram.rearrange("(h p b k) -> p h b k", h=2, p=128, b=BL),
            in_=bq4)
        bp8 = state.tile([128, T * K], F32)
        nc.sync.dma_start(
            out=bp8[0:BL, :].rearrange("p (t k) -> p t k", t=T),
            in_=bp_dram.rearrange("(t b k) -> b t k", t=T, b=BL))
        bp8_r = bp8[:].rearrange("p (t k) -> p t k", t=T)

        # ---- P7: end-tag + backtrace ----
        best = state.tile([128, T], F32)
        pesel = vbig.tile([128, T * K], F32, tag="pesel")
        nc.vector.tensor_tensor(
            out=pesel[0:BL, :].rearrange("p (t k) -> p t k", t=T),
            in0=preH_r[0:BL, :, :],
            in1=endSel[0:BL, :].unsqueeze(2).broadcast_to((BL, T, K)),
            op=AL.mult)
        pe = vbig.tile([128, K], F32, tag="pe")
        nc.vector.tensor_reduce(
            out=pe[0:BL, :],
            in_=pesel[0:BL, :].rearrange("p (t k) -> p k t", t=T),
            axis=AX.X, op=AL.max)
        mvE = vit_p.tile([128, 1], F32, tag="mvE")
        nc.vector.tensor_reduce(out=mvE[0:BL, :], in_=pe[0:BL, :], axis=AX.X, op=AL.max)
        eqE = vit_p.tile([128, K], F32, tag="eqE")
        nc.vector.tensor_tensor(out=eqE[0:BL, :], in0=pe[0:BL, :],
                                in1=mvE[0:BL, :].broadcast_to((BL, K)), op=AL.is_equal)
        nc.vector.tensor_mul(out=eqE[0:BL, :], in0=eqE[0:BL, :], in1=iotaD[0:BL, :])
        eT = vit_p.tile([128, 1], F32, tag="eT")
        nc.vector.tensor_reduce(out=eT[0:BL, :], in_=eqE[0:BL, :], axis=AX.X, op=AL.max)
        nc.vector.tensor_scalar(out=best[0:BL, T - 1:T], in0=eT[0:BL, :],
                                scalar1=-1.0, scalar2=9.0, op0=AL.mult, op1=AL.add)

        for t in range(T - 2, -1, -1):
            oh = vit_p.tile([128, K], F32, tag="oh")
            nc.vector.tensor_tensor(
                out=oh[0:BL, :], in0=iotaK[0:BL, :],
                in1=best[0:BL, t + 1:t + 2].broadcast_to((BL, K)), op=AL.is_equal)
            dmy = vit_p.tile([128, K], F32, tag="dmy")
            nc.vector.tensor_mul(out=dmy[0:BL, :], in0=bp8_r[0:BL, t, :], in1=oh[0:BL, :])
            nc.vector.tensor_reduce(out=best[0:BL, t:t + 1], in_=dmy[0:BL, :],
                                    axis=AX.X, op=AL.max)

        bestM = state.tile([128, T], F32)
        nc.vector.tensor_mul(out=bestM[0:BL, :], in0=best[0:BL, :], in1=maskBT[0:BL, :])
        nc.sync.dma_start(out=out_d, in_=bestM[0:BL, :])
        ctx.close()

    nc.compile()
    return nc


_NC_CACHE = None


def _get_nc():
    global _NC_CACHE
    if _NC_CACHE is None:
        _NC_CACHE = _build()
    return _NC_CACHE


TRACE = False
LAST_EXEC_NS = None


def kernel(**inputs) -> np.ndarray:
    global LAST_EXEC_NS
    nc = _get_nc()
    in_maps = [_prep_core(inputs, c) for c in range(NC_)]
    res = run_bass_kernel_spmd(nc, in_maps, list(range(NC_)), trace=TRACE)
    LAST_EXEC_NS = res.exec_time_ns
    out = np.concatenate([res.results[c]['out'] for c in range(NC_)], axis=0)
    return out.astype(np.float32)


if __name__ == '__main__':
    _build()
    print("build ok")


# revision 25
# speedup vs baseline: 1.5990x; 1.3194x over previous
"""BiLSTM-CRF (Viterbi decode) Trainium2 Bass kernel, 8-core data-parallel.

Full inputs in, full outputs out. Batch (64) is sharded 8 ways; each core runs:
  bf16 embedding gather -> input matmuls (gx = x @ Wih^T + b) -> 256-step fused
  fwd+bwd LSTM recurrence -> fc emissions -> Viterbi scan -> batched
  backpointer extraction -> backtrace.

Layout: gate/hidden dims live on SBUF partitions, (t, b) t-major on the free
dim. The LSTM scan folds the gx add into PSUM via an identity matmul and keeps
a minimal serial chain: sig(i,f) -> tanh(g) -> t1 -> c-add -> tanh(c) -> h.
"""

import os
import sys
import types

for _p in ('/opt/trn_rl_repo', '/root/.axon_site'):
    if _p not in sys.path:
        sys.path.insert(0, _p)

import numpy as np
import ml_dtypes

# ---- NTFF profile hook (lets run_bass_kernel_spmd(trace=True) return timings
# under axon; harmless if already registered or unavailable) ----
def _install_ntff_hook():
    try:
        import antenv
        if 'antenv.axon_hooks' in sys.modules:
            return
        from trn_agent_boot.trn_boot import _ntff_profile_via_ctypes
        m = types.ModuleType('antenv.axon_hooks')
        m._hook = _ntff_profile_via_ctypes('/opt/axon/libaxon_pjrt.so')
        m.get_axon_ntff_profile_hook = lambda: m._hook
        m.set_axon_ntff_profile_hook = lambda h: setattr(m, '_hook', h)
        sys.modules['antenv.axon_hooks'] = m
        antenv.axon_hooks = m
    except Exception:
        pass


_install_ntff_hook()

import concourse.bass as bass
import concourse.tile as tile
from concourse import bacc, mybir
from concourse.bass import IndirectOffsetOnAxis
from concourse.bass_utils import run_bass_kernel_spmd

F32 = mybir.dt.float32
BF16 = mybir.dt.bfloat16
I32 = mybir.dt.int32

# Problem dims (hardcoded per contract)
V, E, HS, T, B = 30000, 256, 512, 256, 64
H = HS // 2          # 256 per-direction hidden
G = 4 * H            # 1024 gate rows per direction
K = 10               # tags
NC_ = 8              # cores
BL = B // NC_        # 8 sequences per core
NBT = BL * T         # 2048 (t,b) columns per core, t-major: j = t*8 + b
NSLOT = NBT // 128   # 16 gather slots

# Gate reorder: torch rows [i, f, g, o] -> device order [i, f, o, g]
_PERM = np.concatenate([
    np.arange(0, 2 * H),          # i, f
    np.arange(3 * H, 4 * H),      # o
    np.arange(2 * H, 3 * H),      # g
])

# gx column layout within a timestep: 128 cols = (tile-group, d, mc-within, b)
# groups: IF (d,mc0-3)->pos 0-7, G (d,mc6-7)->pos 8-11... we use
# col = POS(d, mc)*8 + b with POS ordering: IF: d*4 + mc (mc<4) in [0,8);
# O: 8 + d*2 + (mc-4) in [8,12)? NO -- order chosen: IF block cols 0:64,
# G block cols 64:96, O block cols 96:128.
def _gx_pos(d, mc):
    if mc < 4:                    # i, f
        return d * 4 + mc         # 0..7
    if mc >= 6:                   # g
        return 8 + d * 2 + (mc - 6)   # 8..11
    return 12 + d * 2 + (mc - 4)      # o: 12..15


def _bf(x):
    return np.ascontiguousarray(np.asarray(x, np.float32).astype(ml_dtypes.bfloat16))


def _f32(x):
    return np.ascontiguousarray(np.asarray(x, np.float32))


def _pack_w(wih, whh, bih, bhh):
    """Per direction: returns (w_ih[128, 2*8*128], w_hh[...], bias[128, 8]) in
    lhsT tile layout w[p, kc, mc, m] = W[perm[mc*128+m], kc*128+p]."""
    out = []
    for W in (wih, whh):
        Wp = np.asarray(W, np.float32)[_PERM]          # [G, Kdim]
        Kd = Wp.shape[1]
        t = Wp.reshape(8, 128, Kd // 128, 128)          # [mc, m, kc, p]
        t = np.transpose(t, (3, 2, 0, 1))               # [p, kc, mc, m]
        out.append(t.reshape(128, -1))
    b = (np.asarray(bih, np.float32) + np.asarray(bhh, np.float32))[_PERM]
    b = b.reshape(8, 128).T                             # [p, mc]
    return out[0], out[1], b


_EMB_BF_CACHE = None


def _prep_core(inputs, core):
    """Host-side prep of all per-core device inputs."""
    global _EMB_BF_CACHE
    s = slice(core * BL, (core + 1) * BL)
    inp = np.asarray(inputs['inp'])[s]        # [8, 256] int
    n = np.asarray(inputs['n'])[s].astype(np.int64)

    t_idx = np.arange(T)
    mask = t_idx[None, :] < n[:, None]
    rev = np.where(mask, n[:, None] - 1 - t_idx[None, :], t_idx[None, :])
    tok_rev = np.take_along_axis(inp, rev, axis=1)

    def idx_pack(tok):  # [8,256] -> [128, 16] slot layout, t-major j = t*8+b
        flat = np.asarray(tok, np.int64).T.reshape(-1)
        return flat.reshape(NSLOT, 128).T.astype(np.int32).copy()

    wf = _pack_w(inputs['W_ih_f'], inputs['W_hh_f'], inputs['b_ih_f'], inputs['b_hh_f'])
    wb = _pack_w(inputs['W_ih_b'], inputs['W_hh_b'], inputs['b_ih_b'], inputs['b_hh_b'])
    w_ih = _bf(np.concatenate([wf[0], wb[0]], axis=1))   # [128, 2*2048]
    w_hh = _bf(np.concatenate([wf[1], wb[1]], axis=1))
    bias = _f32(np.concatenate([wf[2], wb[2]], axis=1))  # [128, 16] (d, mc)

    fcw = np.asarray(inputs['fc_w'], np.float32)         # [10, 512]
    fcw_t = fcw.T.reshape(4, 128, K).transpose(1, 0, 2).reshape(128, 4 * K)
    fcbR = np.tile(np.asarray(inputs['fc_b'], np.float32)[None, :], (128, 1))

    trans = np.asarray(inputs['transition'], np.float32)[:K, :K]  # [prev, cur]
    transR = np.tile(trans.T.reshape(1, K * K), (128, 1)).copy()  # [p, cur*10+prev]

    iotaD = np.tile((9.0 - np.arange(K, dtype=np.float32))[None, :], (128, 1))
    iotaK = np.tile(np.arange(K, dtype=np.float32)[None, :], (128, 1))

    # validT[p, h*8+b] = ((h*128+p)+1 < n_b); ivT[p, (h,b,k)] = k*(1-valid)
    tt = (np.arange(256).reshape(2, 128).T)[:, :, None]            # [p, h, 1]
    validT = (tt + 1 < n[None, None, :]).astype(np.float32)        # [p, h, b]
    ivT = (1.0 - validT)[:, :, :, None] * np.arange(K, dtype=np.float32)[None, None, None, :]
    endSel = np.zeros((128, T), np.float32)
    endSel[np.arange(BL), (n - 1)] = 1.0
    # final-mask in Viterbi partition layout p = (b, grp); t = grp*16+tw
    t_all = np.arange(T).reshape(16, 16)                 # [grp, tw]
    maskQ = (t_all[None] < n[:, None, None]).astype(np.float32).reshape(128, 16)

    # hb re-reversal gather rows: out col j'=(t,b) <- hb_dram row rev[b,t]*8 + b
    hb_rows = (rev.T * 8 + np.arange(BL)[None, :]).reshape(-1)     # t-major
    hb_off = hb_rows.reshape(NSLOT, 128).T.astype(np.int32).copy()

    if _EMB_BF_CACHE is None:
        _EMB_BF_CACHE = _bf(inputs['emb'])

    return {
        'emb': _EMB_BF_CACHE,
        'xidx': idx_pack(inp),
        'xridx': idx_pack(tok_rev),
        'w_ih': w_ih, 'w_hh': w_hh, 'bias32': bias,
        'fcw': _bf(fcw_t), 'fcbR': fcbR,
        'ident_bf': _bf(np.eye(128, dtype=np.float32)),
        'transR': transR, 'iotaD': iotaD, 'iotaK': iotaK,
        'validT': _f32(validT.reshape(128, 16)),
        'ivT': _f32(ivT.reshape(128, 160)),
        'endSel': endSel, 'maskQ': _f32(maskQ),
        'hb_off': hb_off,
    }


# ----------------------------------------------------------------------------
# Device kernel
# ----------------------------------------------------------------------------

def _build():
    nc = bacc.Bacc("TRN2", target_bir_lowering=False, debug=False,
                   num_devices=NC_)

    d_in = {}
    def din(name, shape, dt):
        d_in[name] = nc.dram_tensor(name, list(shape), dt, kind="ExternalInput").ap()
        return d_in[name]

    emb_d = din('emb', [V, E], BF16)
    xidx_d = din('xidx', [128, NSLOT], I32)
    xridx_d = din('xridx', [128, NSLOT], I32)
    wih_d = din('w_ih', [128, 2 * 2 * 8 * 128], BF16)
    whh_d = din('w_hh', [128, 2 * 2 * 8 * 128], BF16)
    bias_d = din('bias32', [128, 16], F32)
    fcw_d = din('fcw', [128, 4 * K], BF16)
    fcb_d = din('fcbR', [128, K], F32)
    idbf_d = din('ident_bf', [128, 128], BF16)
    trans_d = din('transR', [128, K * K], F32)
    iotaD_d = din('iotaD', [128, K], F32)
    iotaK_d = din('iotaK', [128, K], F32)
    validT_d = din('validT', [128, 16], F32)
    ivT_d = din('ivT', [128, 160], F32)
    endSel_d = din('endSel', [128, T], F32)
    maskQ_d = din('maskQ', [128, 16], F32)
    hboff_d = din('hb_off', [128, NSLOT], I32)

    out_d = nc.dram_tensor('out', [BL, T], F32, kind="ExternalOutput").ap()

    SIG = mybir.ActivationFunctionType.Sigmoid
    TANH = mybir.ActivationFunctionType.Tanh
    AL = mybir.AluOpType
    AX = mybir.AxisListType

    with tile.TileContext(nc) as tc:
        from contextlib import ExitStack
        ctx = ExitStack()
        cpool = ctx.enter_context(tc.tile_pool(name="consts", bufs=1))
        state = ctx.enter_context(tc.tile_pool(name="state", bufs=1))
        gather_p = ctx.enter_context(tc.tile_pool(name="gather", bufs=2))
        scratch = ctx.enter_context(tc.tile_pool(name="scratch", bufs=3))
        dram_p = ctx.enter_context(tc.tile_pool(name="dram", bufs=1, space="DRAM"))

        hb_dram_t = dram_p.tile([NBT, H], BF16)
        feats_dram_t = dram_p.tile([BL * T * K], F32)
        pre_dram_t = dram_p.tile([T * BL * K], F32)
        bp2dram_t = dram_p.tile([K + BL * T * K], F32)
        g2dram_t = dram_p.tile([128 * K], F32)
        bnddram_t = dram_p.tile([128], F32)
        hb_dram = hb_dram_t[:]
        feats_dram = feats_dram_t[:]
        pre_dram = pre_dram_t[:]
        bp2dram = bp2dram_t[:]
        g2dram = g2dram_t[:]
        bnddram = bnddram_t[:]

        def load_const(dram, shape, dt, tag):
            t = cpool.tile(shape, dt, tag=tag)
            nc.sync.dma_start(t[:], dram)
            return t

        wih = load_const(wih_d[:], [128, 4096], BF16, tag='wih')
        whh = load_const(whh_d[:], [128, 4096], BF16, tag='whh')
        bias = load_const(bias_d[:], [128, 16], F32, tag='bias')
        fcw = load_const(fcw_d[:], [128, 4 * K], BF16, tag='fcw')
        fcbR = load_const(fcb_d[:], [128, K], F32, tag='fcbR')
        ident_bf = load_const(idbf_d[:], [128, 128], BF16, tag='ident_bf')
        transR = load_const(trans_d[:], [128, K * K], F32, tag='transR')
        iotaD = load_const(iotaD_d[:], [128, K], F32, tag='iotaD')
        iotaK = load_const(iotaK_d[:], [128, K], F32, tag='iotaK')
        validT = load_const(validT_d[:], [128, 16], F32, tag='validT')
        ivT = load_const(ivT_d[:], [128, 160], F32, tag='ivT')
        endSel = load_const(endSel_d[:], [128, T], F32, tag='endSel')
        maskQ = load_const(maskQ_d[:], [128, 16], F32, tag='maskQ')
        xidx = load_const(xidx_d[:], [128, NSLOT], I32, tag='xidx')
        xridx = load_const(xridx_d[:], [128, NSLOT], I32, tag='xridx')
        hboff = load_const(hboff_d[:], [128, NSLOT], I32, tag='hboff')

        wih_r = wih[:].rearrange("p (d kc mc m) -> p d kc mc m", d=2, kc=2, mc=8)
        whh_r = whh[:].rearrange("p (d kc mc m) -> p d kc mc m", d=2, kc=2, mc=8)
        fcw_r = fcw[:].rearrange("p (c k) -> p c k", c=4)

        # ---- P1: embedding gather (bf16) + transpose to x^T (E on parts) ----
        gx_ctx = ExitStack()
        gxpool = gx_ctx.enter_context(tc.tile_pool(name="gxp", bufs=1))
        x_ctx = ExitStack()
        xpool = x_ctx.enter_context(tc.tile_pool(name="xp", bufs=1))
        ps_tr = x_ctx.enter_context(tc.tile_pool(name="ps_tr", bufs=2, space="PSUM"))
        ps_mm = x_ctx.enter_context(tc.tile_pool(name="ps_mm", bufs=2, space="PSUM"))
        x_bf = xpool.tile([128, 2 * 2 * NBT], BF16)   # [p, dir, ec, bt']
        xbf_r = x_bf[:].rearrange("p (d e n) -> p d e n", d=2, e=2)
        for d, idxt in ((0, xidx), (1, xridx)):
            for s_ in range(NSLOT):
                xs = gather_p.tile([128, E], BF16, tag="xslot")
                nc.gpsimd.indirect_dma_start(
                    out=xs[:], out_offset=None, in_=emb_d,
                    in_offset=IndirectOffsetOnAxis(ap=idxt[:, s_:s_ + 1], axis=0),
                )
                for ec in range(2):
                    pt = ps_tr.tile([128, 128], BF16, tag="ptr")
                    nc.tensor.transpose(out=pt[:], in_=xs[:, ec * 128:(ec + 1) * 128],
                                        identity=ident_bf[:])
                    nc.vector.tensor_copy(
                        out=xbf_r[:, d, ec, s_ * 128:(s_ + 1) * 128], in_=pt[:])

        # ---- P2: gx = x @ Wih^T + bias (both dirs), bf16, scan layout ----
        # gx cols: t*128 + pos(d,mc)*8 + b  (IF block 0:64, G 64:96, O 96:128)
        gx = gxpool.tile([128, T * 128], BF16)
        gx_v = gx[:].rearrange("p (t c) -> p t c", t=T)
        gx_tb = gx[:].rearrange("p (t c b) -> p t c b", t=T, c=16)
        NB = NBT // 512
        for nb in range(NB):
            t0 = nb * 64
            for d in range(2):
                for mc in range(8):
                    pm = ps_mm.tile([128, 512], F32, tag="pmm")
                    for kc in range(2):
                        nc.tensor.matmul(
                            out=pm[:], lhsT=wih_r[:, d, kc, mc, :],
                            rhs=xbf_r[:, d, kc, nb * 512:(nb + 1) * 512],
                            start=(kc == 0), stop=(kc == 1))
                    pos = _gx_pos(d, mc)
                    nc.vector.tensor_scalar(
                        out=gx_tb[:, t0:t0 + 64, pos, :],
                        in0=pm[:].rearrange("p (t b) -> p t b", t=64),
                        scalar1=bias[:, d * 8 + mc:d * 8 + mc + 1], scalar2=None,
                        op0=AL.add)

        x_ctx.close()

        # ---- P3: fused fwd+bwd LSTM scan ----
        scan_ctx = ExitStack()
        ps_if = scan_ctx.enter_context(tc.tile_pool(name="ps_if", bufs=2, space="PSUM"))
        ps_g = scan_ctx.enter_context(tc.tile_pool(name="ps_g", bufs=2, space="PSUM"))
        ps_o = scan_ctx.enter_context(tc.tile_pool(name="ps_o", bufs=2, space="PSUM"))
        hall = state.tile([128, 2 * 2 * (T + 1) * BL], BF16)  # [p, d, kc, t, b]
        hall_r = hall[:].rearrange("p (d kc t b) -> p d kc t b", d=2, kc=2, t=T + 1)
        cst = state.tile([128, 2 * 2 * BL], F32)              # [p, d, kc, b]
        cst_r = cst[:].rearrange("p (d kc b) -> p d kc b", d=2, kc=2)
        nc.vector.memset(hall_r[:, :, :, 0, :], 0.0)
        nc.vector.memset(cst[:], 0.0)

        # weight tile order per group: IF: (d, mc 0..3), G: (d, mc 6..7), O: (d, mc 4..5)
        for t in range(T):
            pif = ps_if.tile([128, 64], F32, tag="pif")    # (d, mc0-3, b)
            pg = ps_g.tile([128, 32], F32, tag="pg")       # (d, g0-1, b)
            po = ps_o.tile([128, 32], F32, tag="po")       # (d, o0-1, b)
            pif_r = pif[:].rearrange("p (d m b) -> p d m b", d=2, m=4)
            pg_r = pg[:].rearrange("p (d m b) -> p d m b", d=2, m=2)
            po_r = po[:].rearrange("p (d m b) -> p d m b", d=2, m=2)
            # IF group
            nc.tensor.matmul(out=pif[:], lhsT=ident_bf[:], rhs=gx_v[:, t, 0:64],
                             start=True, stop=False)
            for d in range(2):
                for mi in range(4):
                    for kc in range(2):
                        nc.tensor.matmul(
                            out=pif_r[:, d, mi, :], lhsT=whh_r[:, d, kc, mi, :],
                            rhs=hall_r[:, d, kc, t, :],
                            start=False, stop=(kc == 1))
            # G group
            nc.tensor.matmul(out=pg[:], lhsT=ident_bf[:], rhs=gx_v[:, t, 64:96],
                             start=True, stop=False)
            for d in range(2):
                for mi in range(2):
                    for kc in range(2):
                        nc.tensor.matmul(
                            out=pg_r[:, d, mi, :], lhsT=whh_r[:, d, kc, 6 + mi, :],
                            rhs=hall_r[:, d, kc, t, :],
                            start=False, stop=(kc == 1))
            # O group
            nc.tensor.matmul(out=po[:], lhsT=ident_bf[:], rhs=gx_v[:, t, 96:128],
                             start=True, stop=False)
            for d in range(2):
                for mi in range(2):
                    for kc in range(2):
                        nc.tensor.matmul(
                            out=po_r[:, d, mi, :], lhsT=whh_r[:, d, kc, 4 + mi, :],
                            rhs=hall_r[:, d, kc, t, :],
                            start=False, stop=(kc == 1))

            sigIF = scratch.tile([128, 64], F32, tag="sigIF")
            sif_r = sigIF[:].rearrange("p (d m b) -> p d m b", d=2, m=4)
            nc.scalar.activation(out=sigIF[:], in_=pif[:], func=SIG)
            tg = scratch.tile([128, 32], F32, tag="tg")
            nc.scalar.activation(out=tg[:], in_=pg[:], func=TANH)
            so = scratch.tile([128, 32], F32, tag="so")
            nc.scalar.activation(out=so[:], in_=po[:], func=SIG)

            cf = scratch.tile([128, 32], F32, tag="cf")
            cf_r = cf[:].rearrange("p (d c b) -> p d c b", d=2, c=2)
            nc.vector.tensor_mul(out=cf_r[:, :, :, :], in0=sif_r[:, :, 2:4, :],
                                 in1=cst_r[:, :, :, :])
            t1 = scratch.tile([128, 32], F32, tag="t1")
            t1_r = t1[:].rearrange("p (d c b) -> p d c b", d=2, c=2)
            nc.vector.tensor_mul(out=t1_r[:, :, :, :], in0=sif_r[:, :, 0:2, :],
                                 in1=tg[:].rearrange("p (d c b) -> p d c b", d=2, c=2))
            nc.vector.tensor_add(out=cst[:], in0=cf[:], in1=t1[:])
            tc_ = scratch.tile([128, 32], F32, tag="tc")
            nc.scalar.activation(out=tc_[:], in_=cst[:], func=TANH)
            nc.vector.tensor_mul(
                out=hall_r[:, :, :, t + 1, :],
                in0=so[:].rearrange("p (d c b) -> p d c b", d=2, c=2),
                in1=tc_[:].rearrange("p (d c b) -> p d c b", d=2, c=2))

        gx_ctx.close()
        scan_ctx.close()

        # ---- P4: hb re-reversal (DRAM bounce + indirect gather + transpose),
        #          then fc emissions; all (t, b) t-major ----
        p4_ctx = ExitStack()
        ps_tr = p4_ctx.enter_context(tc.tile_pool(name="ps_tr2", bufs=2, space="PSUM"))
        ps_fc = p4_ctx.enter_context(tc.tile_pool(name="ps_fc", bufs=2, space="PSUM"))
        for s_ in range(NSLOT):
            hbs = gather_p.tile([128, H], BF16, tag="hbs")
            for ec in range(2):
                pt = ps_tr.tile([128, 128], BF16, tag="ptr")
                nc.tensor.transpose(
                    out=pt[:],
                    in_=hall_r[:, 1, ec, 1 + s_ * 16:1 + (s_ + 1) * 16, :],
                    identity=ident_bf[:])
                nc.vector.tensor_copy(out=hbs[:, ec * 128:(ec + 1) * 128], in_=pt[:])
            nc.sync.dma_start(out=hb_dram[s_ * 128:(s_ + 1) * 128, :], in_=hbs[:])
        hbT = state.tile([128, 2 * NBT], BF16)   # [p(hid), kc, bt']
        hbT_r = hbT[:].rearrange("p (kc n) -> p kc n", kc=2)
        for s_ in range(NSLOT):
            hs = gather_p.tile([128, H], BF16, tag="hslot")
            nc.gpsimd.indirect_dma_start(
                out=hs[:], out_offset=None, in_=hb_dram,
                in_offset=IndirectOffsetOnAxis(ap=hboff[:, s_:s_ + 1], axis=0))
            for ec in range(2):
                pt = ps_tr.tile([128, 128], BF16, tag="ptr")
                nc.tensor.transpose(out=pt[:], in_=hs[:, ec * 128:(ec + 1) * 128],
                                    identity=ident_bf[:])
                nc.vector.tensor_copy(out=hbT_r[:, ec, s_ * 128:(s_ + 1) * 128], in_=pt[:])

        feats_sb = state.tile([128, 16 * K], F32)   # [p=(tw,b), mt, k]
        feats_r = feats_sb[:].rearrange("p (m k) -> p m k", m=16)
        for mt in range(16):
            pf = ps_fc.tile([128, K], F32, tag="pfc")
            for c4 in range(4):
                if c4 < 2:
                    lhs = hall_r[:, 0, c4, 1 + mt * 16:1 + (mt + 1) * 16, :]
                else:
                    lhs = hbT_r[:, c4 - 2, mt * 128:(mt + 1) * 128]
                nc.tensor.matmul(out=pf[:], lhsT=lhs, rhs=fcw_r[:, c4, :],
                                 start=(c4 == 0), stop=(c4 == 3))
            nc.vector.tensor_tensor(out=feats_r[:, mt, :], in0=pf[:],
                                    in1=fcbR[:, :], op=AL.add)
        p4_ctx.close()

        # relayout feats -> [b partitions, (t, k)]; p=(tw, b), t = mt*16+tw
        # bounce stored (tw, b, mt, k); load permutes to (b, mt, tw, k) = (b, t, k)
        nc.sync.dma_start(
            out=feats_dram.rearrange("(tw b mt k) -> (tw b) mt k", tw=16, b=BL, mt=16),
            in_=feats_r[:, :, :])
        feats8 = state.tile([128, T * K], F32)
        nc.sync.dma_start(
            out=feats8[0:BL, :].rearrange("p (mt tw k) -> p mt tw k", mt=16, tw=16),
            in_=feats_dram.rearrange("(tw b mt k) -> b mt tw k", tw=16, b=BL, mt=16))
        f8_r = feats8[:].rearrange("p (t k) -> p t k", t=T)

        # ==== P5: Viterbi forward scan (exact serial, batch on partitions) ====
        vit_p = ctx.enter_context(tc.tile_pool(name="vit", bufs=4))
        vbig = ctx.enter_context(tc.tile_pool(name="vbig", bufs=1))
        preH = state.tile([128, T * K], F32)
        preH_r = preH[:].rearrange("p (t k) -> p t k", t=T)
        nc.vector.tensor_copy(out=preH_r[0:BL, 0, :], in_=f8_r[0:BL, 0, :])
        for t in range(1, T):
            s1 = vit_p.tile([128, K * K], F32, tag="s1")
            nc.vector.tensor_tensor(
                out=s1[0:BL, :].rearrange("p (c q) -> p c q", c=K),
                in0=preH_r[0:BL, t - 1, :].unsqueeze(1)
                    .broadcast_to((BL, K, K)),
                in1=transR[0:BL, :].rearrange("p (c q) -> p c q", c=K), op=AL.add)
            m1 = vit_p.tile([128, K], F32, tag="m1")
            nc.vector.tensor_reduce(
                out=m1[0:BL, :], in_=s1[0:BL, :].rearrange("p (c q) -> p c q", c=K),
                axis=AX.X, op=AL.max)
            nc.vector.tensor_tensor(out=preH_r[0:BL, t, :], in0=m1[0:BL, :],
                                    in1=f8_r[0:BL, t, :], op=AL.add)

        # ==== P6: batched backpointer extraction (exact) ====
        nc.sync.dma_start(
            out=pre_dram.rearrange("(t b k) -> b t k", t=T, b=BL),
            in_=preH_r[0:BL, :, :])
        preT = vbig.tile([128, 2 * BL * K], F32, tag="preT")
        preT_r = preT[:].rearrange("p (h b k) -> p h b k", h=2, b=BL)
        nc.sync.dma_start(
            out=preT_r[:, :, :, :],
            in_=pre_dram.rearrange("(h p b k) -> p h b k", h=2, p=128, b=BL))

        HB = 2 * BL
        preT_hb = preT[:].rearrange("p (hb k) -> p hb k", k=K)
        sX = vbig.tile([128, 2 * BL * K * K], F32, tag="sX")
        sX_r = sX[:].rearrange("p (hb c q) -> p hb c q", hb=HB, c=K)
        nc.vector.tensor_tensor(
            out=sX_r[:, :, :, :],
            in0=preT_hb.unsqueeze(2).broadcast_to((128, HB, K, K)),
            in1=transR[:, :].rearrange("p (c q) -> p c q", c=K).unsqueeze(1)
                .broadcast_to((128, HB, K, K)),
            op=AL.add)
        mX = vbig.tile([128, 2 * BL * K], F32, tag="mX")
        mX_r = mX[:].rearrange("p (hb c) -> p hb c", hb=HB)
        nc.vector.tensor_reduce(out=mX_r[:, :, :], in_=sX_r[:, :, :, :],
                                axis=AX.X, op=AL.max)
        nc.vector.tensor_tensor(
            out=sX_r[:, :, :, :], in0=sX_r[:, :, :, :],
            in1=mX_r[:, :, :].unsqueeze(3).broadcast_to((128, HB, K, K)),
            op=AL.is_equal)
        nc.vector.tensor_tensor(
            out=sX_r[:, :, :, :], in0=sX_r[:, :, :, :],
            in1=iotaD[:, :].unsqueeze(1).unsqueeze(1).broadcast_to((128, HB, K, K)),
            op=AL.mult)
        bq = vbig.tile([128, 2 * BL * K], F32, tag="bq")
        bq_r = bq[:].rearrange("p (hb c) -> p hb c", hb=HB)
        nc.vector.tensor_reduce(out=bq_r[:, :, :], in_=sX_r[:, :, :, :],
                                axis=AX.X, op=AL.max)
        # bp = 9 - bq ; then pad override: bp*valid + iota_cur*(1-valid)
        nc.vector.tensor_scalar(out=bq[:], in0=bq[:], scalar1=-1.0, scalar2=9.0,
                                op0=AL.mult, op1=AL.add)
        nc.vector.tensor_tensor(
            out=bq_r[:, :, :], in0=bq_r[:, :, :],
            in1=validT[:, :].unsqueeze(2).broadcast_to((128, HB, K)),
            op=AL.mult)
        nc.vector.tensor_tensor(
            out=bq_r[:, :, :], in0=bq_r[:, :, :],
            in1=ivT[:, :].rearrange("p (hb k) -> p hb k", k=K),
            op=AL.add)
        # store bp (bq row t_idx = bp at time t_idx+1) into flat (b, t, k)
        # order shifted forward by one step via a K-element front pad; the
        # shifted reload then yields bpQ[(b,grp), (tw, c)] = m_t (bp at t),
        # with t=0 slots landing on iota (identity), as the backtrace wants.
        iK = vit_p.tile([128, K], F32, tag="iK")
        nc.vector.tensor_copy(out=iK[:], in_=iotaK[:, :])
        nc.sync.dma_start(out=bp2dram[0:K].rearrange("(o x) -> o x", o=1),
                          in_=iK[0:1, :])
        bq4 = bq[:].rearrange("p (h b k) -> p h b k", h=2, b=BL)
        bp2v = bp2dram[K:].rearrange("(b h2 p k) -> h2 p b k", b=BL, h2=2, p=128)
        for h_ in range(2):
            nc.sync.dma_start(out=bp2v[h_], in_=bq4[:, h_, :, :])
        bpQ = vbig.tile([128, 160], F32, tag="bpQ")
        nc.sync.dma_start(out=bpQ[:],
                          in_=bp2dram[0:128 * 160].rearrange("(p x) -> p x", p=128))

        # ==== end-tag (exact, from preH) ====
        pesel = vbig.tile([128, T * K], F32, tag="pesel")
        nc.vector.tensor_tensor(
            out=pesel[0:BL, :].rearrange("p (t k) -> p t k", t=T),
            in0=preH_r[0:BL, :, :],
            in1=endSel[0:BL, :].unsqueeze(2).broadcast_to((BL, T, K)),
            op=AL.mult)
        pe = vbig.tile([128, K], F32, tag="pe")
        nc.vector.tensor_reduce(
            out=pe[0:BL, :],
            in_=pesel[0:BL, :].rearrange("p (t k) -> p k t", t=T),
            axis=AX.X, op=AL.max)
        mvE = vit_p.tile([128, 1], F32, tag="mvE")
        nc.vector.tensor_reduce(out=mvE[0:BL, :], in_=pe[0:BL, :], axis=AX.X, op=AL.max)
        eqE = vit_p.tile([128, K], F32, tag="eqE")
        nc.vector.tensor_tensor(out=eqE[0:BL, :], in0=pe[0:BL, :],
                                in1=mvE[0:BL, :].broadcast_to((BL, K)), op=AL.is_equal)
        nc.vector.tensor_mul(out=eqE[0:BL, :], in0=eqE[0:BL, :], in1=iotaD[0:BL, :])
        endT = vit_p.tile([128, 1], F32, tag="endT")
        nc.vector.tensor_reduce(out=endT[0:BL, :], in_=eqE[0:BL, :], axis=AX.X, op=AL.max)
        nc.vector.tensor_scalar(out=endT[0:BL, :], in0=endT[0:BL, :],
                                scalar1=-1.0, scalar2=9.0, op0=AL.mult, op1=AL.add)

        # ==== backtrace via map composition ====
        bpQ4 = bpQ[:].rearrange("p (i w k) -> p i w k", i=4, w=4)

        def compose4(a_view, b_view, tag):
            # out[p, i, k] = a[p, i, b[p, i, k]] for 4 quads per partition
            e4 = vit_p.tile([128, 400], F32, tag="e4")
            e4_r = e4[:].rearrange("p (i k j) -> p i k j", i=4, k=K)
            nc.vector.tensor_tensor(
                out=e4_r, in0=b_view.unsqueeze(3).broadcast_to((128, 4, K, K)),
                in1=iotaK[:, :].unsqueeze(1).unsqueeze(1)
                    .broadcast_to((128, 4, K, K)),
                op=AL.is_equal)
            nc.vector.tensor_tensor(
                out=e4_r, in0=e4_r,
                in1=a_view.unsqueeze(2).broadcast_to((128, 4, K, K)), op=AL.mult)
            o = vbig.tile([128, 4 * K], F32, tag=tag)
            o_r = o[:].rearrange("p (i k) -> p i k", i=4)
            nc.vector.tensor_reduce(out=o_r, in_=e4_r, axis=AX.X, op=AL.max)
            return o, o_r

        sk3 = bpQ4[:, :, 3, :]
        sk2, sk2_r = compose4(bpQ4[:, :, 2, :], sk3, "sk2")
        sk1, sk1_r = compose4(bpQ4[:, :, 1, :], sk2_r, "sk1")
        sk0, sk0_r = compose4(bpQ4[:, :, 0, :], sk1_r, "sk0")   # Fq per quad

        def compose1(a_view, b_view, tag):
            # out[p, k] = a[p, b[p, k]]
            e1 = vit_p.tile([128, K * K], F32, tag="e1")
            e1_r = e1[:].rearrange("p (k j) -> p k j", k=K)
            nc.vector.tensor_tensor(
                out=e1_r, in0=b_view.unsqueeze(2).broadcast_to((128, K, K)),
                in1=iotaK[:, :].unsqueeze(1).broadcast_to((128, K, K)),
                op=AL.is_equal)
            nc.vector.tensor_tensor(
                out=e1_r, in0=e1_r,
                in1=a_view.unsqueeze(1).broadcast_to((128, K, K)), op=AL.mult)
            o = vbig.tile([128, K], F32, tag=tag)
            nc.vector.tensor_reduce(out=o[:], in_=e1_r, axis=AX.X, op=AL.max)
            return o

        sg3 = sk0_r[:, 3, :]
        sg2 = compose1(sk0_r[:, 2, :], sg3, "sg2")
        sg1 = compose1(sk0_r[:, 1, :], sg2[:], "sg1")
        sg0 = compose1(sk0_r[:, 0, :], sg1[:], "sg0")           # G per grp

        # G relayout -> [b, (grp, k)]
        nc.sync.dma_start(out=g2dram.rearrange("(p x) -> p x", p=128), in_=sg0[:])
        Gs = vbig.tile([128, 16 * K], F32, tag="Gs")
        nc.sync.dma_start(out=Gs[0:BL, :], in_=g2dram.rearrange("(b x) -> b x", b=BL))
        Gs_r = Gs[:].rearrange("b (g k) -> b g k", g=16)

        # serial grp chase: bnd_15 = end; bnd_{g-1} = G_g[bnd_g]
        bndAll = vbig.tile([128, 16], F32, tag="bndAll")
        nc.vector.tensor_copy(out=bndAll[0:BL, 15:16], in_=endT[0:BL, :])
        for g in range(15, 0, -1):
            eqA = vit_p.tile([128, K], F32, tag="eqA")
            nc.vector.tensor_tensor(
                out=eqA[0:BL, :], in0=iotaK[0:BL, :],
                in1=bndAll[0:BL, g:g + 1].broadcast_to((BL, K)), op=AL.is_equal)
            nc.vector.tensor_mul(out=eqA[0:BL, :], in0=eqA[0:BL, :], in1=Gs_r[0:BL, g, :])
            nc.vector.tensor_reduce(out=bndAll[0:BL, g - 1:g], in_=eqA[0:BL, :],
                                    axis=AX.X, op=AL.max)

        # bnd relayout -> [(b,grp), 1]
        nc.sync.dma_start(out=bnddram.rearrange("(b x) -> b x", b=BL), in_=bndAll[0:BL, :])
        bndP = vbig.tile([128, 1], F32, tag="bndP")
        nc.sync.dma_start(out=bndP[:], in_=bnddram.rearrange("(p x) -> p x", p=128))

        # sgq[p, i, j]: entry map for quad i = sg_{i+1} (sg_4 = Id)
        sgq = vbig.tile([128, 4 * K], F32, tag="sgq")
        nc.vector.tensor_copy(out=sgq[:, 0:K], in_=sg1[:])
        nc.vector.tensor_copy(out=sgq[:, K:2 * K], in_=sg2[:])
        nc.vector.tensor_copy(out=sgq[:, 2 * K:3 * K], in_=sk0_r[:, 3, :])
        nc.vector.tensor_copy(out=sgq[:, 3 * K:4 * K], in_=iotaK[:, :])
        # qbEntry[p, i] = sgq_i[bnd]
        eqi = vit_p.tile([128, 4 * K], F32, tag="eqi")
        eqi_r = eqi[:].rearrange("p (i j) -> p i j", i=4)
        nc.vector.tensor_tensor(
            out=eqi_r,
            in0=iotaK[:, :].unsqueeze(1).broadcast_to((128, 4, K)),
            in1=bndP[:].unsqueeze(1).broadcast_to((128, 4, K)), op=AL.is_equal)
        nc.vector.tensor_tensor(
            out=eqi_r, in0=eqi_r,
            in1=sgq[:].rearrange("p (i j) -> p i j", i=4), op=AL.mult)
        qbE = vbig.tile([128, 4], F32, tag="qbE")
        nc.vector.tensor_reduce(out=qbE[:], in_=eqi_r, axis=AX.X, op=AL.max)

        # skq[p, i, w, j]: within-quad entry maps = sk_{w+1} (sk_4 = Id)
        skq = vbig.tile([128, 160], F32, tag="skq")
        skq_r = skq[:].rearrange("p (i w j) -> p i w j", i=4, w=4)
        nc.vector.tensor_copy(out=skq_r[:, :, 0, :], in_=sk1_r)
        nc.vector.tensor_copy(out=skq_r[:, :, 1, :], in_=sk2_r)
        nc.vector.tensor_copy(out=skq_r[:, :, 2, :], in_=bpQ4[:, :, 3, :])
        for _i in range(4):
            nc.vector.tensor_copy(out=skq_r[:, _i, 3, :], in_=iotaK[:, :])
        # tagOut[p, (i, w)] = skq_{i,w}[qbE_i]
        eqt = vit_p.tile([128, 160], F32, tag="eqt")
        eqt_r = eqt[:].rearrange("p (i w j) -> p i w j", i=4, w=4)
        nc.vector.tensor_tensor(
            out=eqt_r,
            in0=iotaK[:, :].unsqueeze(1).unsqueeze(1).broadcast_to((128, 4, 4, K)),
            in1=qbE[:].rearrange("p (i o) -> p i o", i=4).unsqueeze(3)
                .broadcast_to((128, 4, 4, K)),
            op=AL.is_equal)
        nc.vector.tensor_tensor(out=eqt_r, in0=eqt_r, in1=skq_r, op=AL.mult)
        tagOut = vbig.tile([128, 16], F32, tag="tagOut")
        nc.vector.tensor_reduce(out=tagOut[:].rearrange("p (i w) -> p i w", i=4),
                                in_=eqt_r, axis=AX.X, op=AL.max)

        bestQ = vbig.tile([128, 16], F32, tag="bestQ")
        nc.vector.tensor_mul(out=bestQ[:], in0=tagOut[:], in1=maskQ[:, :])
        nc.sync.dma_start(
            out=out_d.rearrange("b (grp tw) -> (b grp) tw", grp=16),
            in_=bestQ[:])
        ctx.close()

    nc.compile()
    return nc


_NC_CACHE = None


def _get_nc():
    global _NC_CACHE
    if _NC_CACHE is None:
        _NC_CACHE = _build()
    return _NC_CACHE


TRACE = False
LAST_EXEC_NS = None


def kernel(**inputs) -> np.ndarray:
    global LAST_EXEC_NS
    nc = _get_nc()
    in_maps = [_prep_core(inputs, c) for c in range(NC_)]
    res = run_bass_kernel_spmd(nc, in_maps, list(range(NC_)), trace=TRACE)
    LAST_EXEC_NS = res.exec_time_ns
    out = np.concatenate([res.results[c]['out'] for c in range(NC_)], axis=0)
    return out.astype(np.float32)


if __name__ == '__main__':
    _build()
    print("build ok")


# revision 27
# speedup vs baseline: 1.6181x; 1.0119x over previous
"""BiLSTM-CRF (Viterbi decode) Trainium2 Bass kernel, 8-core data-parallel.

Full inputs in, full outputs out. Batch (64) is sharded 8 ways; each core runs:
  bf16 embedding gather -> input matmuls (gx = x @ Wih^T + b) -> 256-step fused
  fwd+bwd LSTM recurrence -> fc emissions -> Viterbi scan -> batched
  backpointer extraction -> backtrace.

Layout: gate/hidden dims live on SBUF partitions, (t, b) t-major on the free
dim. The LSTM scan folds the gx add into PSUM via an identity matmul and keeps
a minimal serial chain: sig(i,f) -> tanh(g) -> t1 -> c-add -> tanh(c) -> h.
"""

import os
import sys
import types

for _p in ('/opt/trn_rl_repo', '/root/.axon_site'):
    if _p not in sys.path:
        sys.path.insert(0, _p)

import numpy as np
import ml_dtypes

# ---- NTFF profile hook (lets run_bass_kernel_spmd(trace=True) return timings
# under axon; harmless if already registered or unavailable) ----
def _install_ntff_hook():
    try:
        import antenv
        if 'antenv.axon_hooks' in sys.modules:
            return
        from trn_agent_boot.trn_boot import _ntff_profile_via_ctypes
        m = types.ModuleType('antenv.axon_hooks')
        m._hook = _ntff_profile_via_ctypes('/opt/axon/libaxon_pjrt.so')
        m.get_axon_ntff_profile_hook = lambda: m._hook
        m.set_axon_ntff_profile_hook = lambda h: setattr(m, '_hook', h)
        sys.modules['antenv.axon_hooks'] = m
        antenv.axon_hooks = m
    except Exception:
        pass


_install_ntff_hook()

import concourse.bass as bass
import concourse.tile as tile
from concourse import bacc, mybir
from concourse.bass import IndirectOffsetOnAxis
from concourse.bass_utils import run_bass_kernel_spmd

F32 = mybir.dt.float32
BF16 = mybir.dt.bfloat16
I32 = mybir.dt.int32

# Problem dims (hardcoded per contract)
V, E, HS, T, B = 30000, 256, 512, 256, 64
H = HS // 2          # 256 per-direction hidden
G = 4 * H            # 1024 gate rows per direction
K = 10               # tags
NC_ = 8              # cores
BL = B // NC_        # 8 sequences per core
NBT = BL * T         # 2048 (t,b) columns per core, t-major: j = t*8 + b
NSLOT = NBT // 128   # 16 gather slots

# Gate reorder: torch rows [i, f, g, o] -> device order [i, f, o, g]
_PERM = np.concatenate([
    np.arange(0, 2 * H),          # i, f
    np.arange(3 * H, 4 * H),      # o
    np.arange(2 * H, 3 * H),      # g
])

# gx column layout within a timestep: 128 cols = (tile-group, d, mc-within, b)
# groups: IF (d,mc0-3)->pos 0-7, G (d,mc6-7)->pos 8-11... we use
# col = POS(d, mc)*8 + b with POS ordering: IF: d*4 + mc (mc<4) in [0,8);
# O: 8 + d*2 + (mc-4) in [8,12)? NO -- order chosen: IF block cols 0:64,
# G block cols 64:96, O block cols 96:128.
def _gx_pos(d, mc):
    if mc < 4:                    # i, f
        return d * 4 + mc         # 0..7
    if mc >= 6:                   # g
        return 8 + d * 2 + (mc - 6)   # 8..11
    return 12 + d * 2 + (mc - 4)      # o: 12..15


def _bf(x):
    return np.ascontiguousarray(np.asarray(x, np.float32).astype(ml_dtypes.bfloat16))


def _f32(x):
    return np.ascontiguousarray(np.asarray(x, np.float32))


def _pack_w(wih, whh, bih, bhh):
    """Per direction: returns (w_ih[128, 2*8*128], w_hh[...], bias[128, 8]) in
    lhsT tile layout w[p, kc, mc, m] = W[perm[mc*128+m], kc*128+p]."""
    out = []
    for W in (wih, whh):
        Wp = np.asarray(W, np.float32)[_PERM]          # [G, Kdim]
        Kd = Wp.shape[1]
        t = Wp.reshape(8, 128, Kd // 128, 128)          # [mc, m, kc, p]
        t = np.transpose(t, (3, 2, 0, 1))               # [p, kc, mc, m]
        out.append(t.reshape(128, -1))
    b = (np.asarray(bih, np.float32) + np.asarray(bhh, np.float32))[_PERM]
    b = b.reshape(8, 128).T                             # [p, mc]
    return out[0], out[1], b


_EMB_BF_CACHE = None


def _prep_core(inputs, core):
    """Host-side prep of all per-core device inputs."""
    global _EMB_BF_CACHE
    s = slice(core * BL, (core + 1) * BL)
    inp = np.asarray(inputs['inp'])[s]        # [8, 256] int
    n = np.asarray(inputs['n'])[s].astype(np.int64)

    t_idx = np.arange(T)
    mask = t_idx[None, :] < n[:, None]
    rev = np.where(mask, n[:, None] - 1 - t_idx[None, :], t_idx[None, :])
    tok_rev = np.take_along_axis(inp, rev, axis=1)

    def idx_pack(tok):  # [8,256] -> [128, 16] slot layout, t-major j = t*8+b
        flat = np.asarray(tok, np.int64).T.reshape(-1)
        return flat.reshape(NSLOT, 128).T.astype(np.int32).copy()

    wf = _pack_w(inputs['W_ih_f'], inputs['W_hh_f'], inputs['b_ih_f'], inputs['b_hh_f'])
    wb = _pack_w(inputs['W_ih_b'], inputs['W_hh_b'], inputs['b_ih_b'], inputs['b_hh_b'])
    w_ih = _bf(np.concatenate([wf[0], wb[0]], axis=1))   # [128, 2*2048]
    w_hh = _bf(np.concatenate([wf[1], wb[1]], axis=1))
    bias = _f32(np.concatenate([wf[2], wb[2]], axis=1))  # [128, 16] (d, mc)

    fcw = np.asarray(inputs['fc_w'], np.float32)         # [10, 512]
    fcw_t = fcw.T.reshape(4, 128, K).transpose(1, 0, 2).reshape(128, 4 * K)
    fcbR = np.tile(np.asarray(inputs['fc_b'], np.float32)[None, :], (128, 1))

    trans = np.asarray(inputs['transition'], np.float32)[:K, :K]  # [prev, cur]
    transR = np.tile(trans.T.reshape(1, K * K), (128, 1)).copy()  # [p, cur*10+prev]

    iotaD = np.tile((9.0 - np.arange(K, dtype=np.float32))[None, :], (128, 1))
    iotaK = np.tile(np.arange(K, dtype=np.float32)[None, :], (128, 1))

    # validT[p, h*8+b] = ((h*128+p)+1 < n_b); ivT[p, (h,b,k)] = k*(1-valid)
    tt = (np.arange(256).reshape(2, 128).T)[:, :, None]            # [p, h, 1]
    validT = (tt + 1 < n[None, None, :]).astype(np.float32)        # [p, h, b]
    ivT = (1.0 - validT)[:, :, :, None] * np.arange(K, dtype=np.float32)[None, None, None, :]
    endSel = np.zeros((128, T), np.float32)
    endSel[np.arange(BL), (n - 1)] = 1.0
    # final-mask in Viterbi partition layout p = (b, grp); t = grp*16+tw
    t_all = np.arange(T).reshape(16, 16)                 # [grp, tw]
    maskQ = (t_all[None] < n[:, None, None]).astype(np.float32).reshape(128, 16)

    # hb re-reversal gather rows: out col j'=(t,b) <- hb_dram row rev[b,t]*8 + b
    hb_rows = (rev.T * 8 + np.arange(BL)[None, :]).reshape(-1)     # t-major
    hb_off = hb_rows.reshape(NSLOT, 128).T.astype(np.int32).copy()

    if _EMB_BF_CACHE is None:
        _EMB_BF_CACHE = _bf(inputs['emb'])

    return {
        'emb': _EMB_BF_CACHE,
        'xidx': idx_pack(inp),
        'xridx': idx_pack(tok_rev),
        'w_ih': w_ih, 'w_hh': w_hh, 'bias32': bias,
        'fcw': _bf(fcw_t), 'fcbR': fcbR,
        'ident_bf': _bf(np.eye(128, dtype=np.float32)),
        'transR': transR, 'iotaD': iotaD, 'iotaK': iotaK,
        'validT': _f32(validT.reshape(128, 16)),
        'ivT': _f32(ivT.reshape(128, 160)),
        'endSel': endSel, 'maskQ': _f32(maskQ),
        'hb_off': hb_off,
    }


# ----------------------------------------------------------------------------
# Device kernel
# ----------------------------------------------------------------------------

def _build():
    nc = bacc.Bacc("TRN2", target_bir_lowering=False, debug=False,
                   num_devices=NC_)

    d_in = {}
    def din(name, shape, dt):
        d_in[name] = nc.dram_tensor(name, list(shape), dt, kind="ExternalInput").ap()
        return d_in[name]

    emb_d = din('emb', [V, E], BF16)
    xidx_d = din('xidx', [128, NSLOT], I32)
    xridx_d = din('xridx', [128, NSLOT], I32)
    wih_d = din('w_ih', [128, 2 * 2 * 8 * 128], BF16)
    whh_d = din('w_hh', [128, 2 * 2 * 8 * 128], BF16)
    bias_d = din('bias32', [128, 16], F32)
    fcw_d = din('fcw', [128, 4 * K], BF16)
    fcb_d = din('fcbR', [128, K], F32)
    idbf_d = din('ident_bf', [128, 128], BF16)
    trans_d = din('transR', [128, K * K], F32)
    iotaD_d = din('iotaD', [128, K], F32)
    iotaK_d = din('iotaK', [128, K], F32)
    validT_d = din('validT', [128, 16], F32)
    ivT_d = din('ivT', [128, 160], F32)
    endSel_d = din('endSel', [128, T], F32)
    maskQ_d = din('maskQ', [128, 16], F32)
    hboff_d = din('hb_off', [128, NSLOT], I32)

    out_d = nc.dram_tensor('out', [BL, T], F32, kind="ExternalOutput").ap()

    SIG = mybir.ActivationFunctionType.Sigmoid
    TANH = mybir.ActivationFunctionType.Tanh
    AL = mybir.AluOpType
    AX = mybir.AxisListType

    with tile.TileContext(nc) as tc:
        from contextlib import ExitStack
        ctx = ExitStack()
        cpool = ctx.enter_context(tc.tile_pool(name="consts", bufs=1))
        state = ctx.enter_context(tc.tile_pool(name="state", bufs=1))
        gather_p = ctx.enter_context(tc.tile_pool(name="gather", bufs=2))
        scratch = ctx.enter_context(tc.tile_pool(name="scratch", bufs=3))
        dram_p = ctx.enter_context(tc.tile_pool(name="dram", bufs=1, space="DRAM"))

        hb_dram_t = dram_p.tile([NBT, H], BF16)
        feats_dram_t = dram_p.tile([BL * T * K], F32)
        pre_dram_t = dram_p.tile([T * BL * K], F32)
        bp2dram_t = dram_p.tile([K + BL * T * K], F32)
        g2dram_t = dram_p.tile([128 * K], F32)
        bnddram_t = dram_p.tile([128], F32)
        hb_dram = hb_dram_t[:]
        feats_dram = feats_dram_t[:]
        pre_dram = pre_dram_t[:]
        bp2dram = bp2dram_t[:]
        g2dram = g2dram_t[:]
        bnddram = bnddram_t[:]

        def load_const(dram, shape, dt, tag):
            t = cpool.tile(shape, dt, tag=tag)
            nc.sync.dma_start(t[:], dram)
            return t

        wih = load_const(wih_d[:], [128, 4096], BF16, tag='wih')
        whh = load_const(whh_d[:], [128, 4096], BF16, tag='whh')
        bias = load_const(bias_d[:], [128, 16], F32, tag='bias')
        fcw = load_const(fcw_d[:], [128, 4 * K], BF16, tag='fcw')
        fcbR = load_const(fcb_d[:], [128, K], F32, tag='fcbR')
        ident_bf = load_const(idbf_d[:], [128, 128], BF16, tag='ident_bf')
        transR = load_const(trans_d[:], [128, K * K], F32, tag='transR')
        iotaD = load_const(iotaD_d[:], [128, K], F32, tag='iotaD')
        iotaK = load_const(iotaK_d[:], [128, K], F32, tag='iotaK')
        validT = load_const(validT_d[:], [128, 16], F32, tag='validT')
        ivT = load_const(ivT_d[:], [128, 160], F32, tag='ivT')
        endSel = load_const(endSel_d[:], [128, T], F32, tag='endSel')
        maskQ = load_const(maskQ_d[:], [128, 16], F32, tag='maskQ')
        xidx = load_const(xidx_d[:], [128, NSLOT], I32, tag='xidx')
        xridx = load_const(xridx_d[:], [128, NSLOT], I32, tag='xridx')
        hboff = load_const(hboff_d[:], [128, NSLOT], I32, tag='hboff')

        wih_r = wih[:].rearrange("p (d kc mc m) -> p d kc mc m", d=2, kc=2, mc=8)
        whh_r = whh[:].rearrange("p (d kc mc m) -> p d kc mc m", d=2, kc=2, mc=8)
        fcw_r = fcw[:].rearrange("p (c k) -> p c k", c=4)

        # ---- P1: embedding gather (bf16) + transpose to x^T (E on parts) ----
        gx_ctx = ExitStack()
        gxpool = gx_ctx.enter_context(tc.tile_pool(name="gxp", bufs=1))
        x_ctx = ExitStack()
        xpool = x_ctx.enter_context(tc.tile_pool(name="xp", bufs=1))
        ps_tr = x_ctx.enter_context(tc.tile_pool(name="ps_tr", bufs=2, space="PSUM"))
        ps_mm = x_ctx.enter_context(tc.tile_pool(name="ps_mm", bufs=2, space="PSUM"))
        x_bf = xpool.tile([128, 2 * 2 * NBT], BF16)   # [p, dir, ec, bt']
        xbf_r = x_bf[:].rearrange("p (d e n) -> p d e n", d=2, e=2)
        for s_ in range(NSLOT):
            for d, idxt in ((0, xidx), (1, xridx)):
                xs = gather_p.tile([128, E], BF16, tag="xslot")
                nc.gpsimd.indirect_dma_start(
                    out=xs[:], out_offset=None, in_=emb_d,
                    in_offset=IndirectOffsetOnAxis(ap=idxt[:, s_:s_ + 1], axis=0),
                )
                for ec in range(2):
                    pt = ps_tr.tile([128, 128], BF16, tag="ptr")
                    nc.tensor.transpose(out=pt[:], in_=xs[:, ec * 128:(ec + 1) * 128],
                                        identity=ident_bf[:])
                    nc.vector.tensor_copy(
                        out=xbf_r[:, d, ec, s_ * 128:(s_ + 1) * 128], in_=pt[:])

        # ---- P2: gx = x @ Wih^T + bias (both dirs), bf16, scan layout ----
        # gx layout (pos, t, b): pos = gate group slot; IF pos 0:8, G 8:12,
        # O 12:16; per-(d,mc) block contiguous so bias-add writes are dense
        gx = gxpool.tile([128, 16 * T * BL], BF16)
        gx_p = gx[:].rearrange("p (c t b) -> p c t b", c=16, t=T)
        NB = NBT // 512
        for nb in range(NB):
            t0 = nb * 64
            for d in range(2):
                for mc in range(8):
                    pm = ps_mm.tile([128, 512], F32, tag="pmm")
                    for kc in range(2):
                        nc.tensor.matmul(
                            out=pm[:], lhsT=wih_r[:, d, kc, mc, :],
                            rhs=xbf_r[:, d, kc, nb * 512:(nb + 1) * 512],
                            start=(kc == 0), stop=(kc == 1))
                    pos = _gx_pos(d, mc)
                    nc.vector.tensor_scalar(
                        out=gx_p[:, pos, t0:t0 + 64, :],
                        in0=pm[:].rearrange("p (t b) -> p t b", t=64),
                        scalar1=bias[:, d * 8 + mc:d * 8 + mc + 1], scalar2=None,
                        op0=AL.add)

        x_ctx.close()

        # ---- P3: fused fwd+bwd LSTM scan ----
        scan_ctx = ExitStack()
        ps_if = scan_ctx.enter_context(tc.tile_pool(name="ps_if", bufs=2, space="PSUM"))
        ps_g = scan_ctx.enter_context(tc.tile_pool(name="ps_g", bufs=2, space="PSUM"))
        ps_o = scan_ctx.enter_context(tc.tile_pool(name="ps_o", bufs=2, space="PSUM"))
        ps_hb = scan_ctx.enter_context(tc.tile_pool(name="ps_hb", bufs=2, space="PSUM"))
        hall = state.tile([128, 2 * 2 * (T + 1) * BL], BF16)  # [p, d, kc, t, b]
        hall_r = hall[:].rearrange("p (d kc t b) -> p d kc t b", d=2, kc=2, t=T + 1)
        cst = state.tile([128, 2 * 2 * BL], F32)              # [p, d, kc, b]
        cst_r = cst[:].rearrange("p (d kc b) -> p d kc b", d=2, kc=2)
        nc.vector.memset(hall_r[:, :, :, 0, :], 0.0)
        nc.vector.memset(cst[:], 0.0)

        # weight tile order per group: IF: (d, mc 0..3), G: (d, mc 6..7), O: (d, mc 4..5)
        for t in range(T):
            pif = ps_if.tile([128, 64], F32, tag="pif")    # (d, mc0-3, b)
            pg = ps_g.tile([128, 32], F32, tag="pg")       # (d, g0-1, b)
            po = ps_o.tile([128, 32], F32, tag="po")       # (d, o0-1, b)
            pif_r = pif[:].rearrange("p (d m b) -> p d m b", d=2, m=4)
            pg_r = pg[:].rearrange("p (d m b) -> p d m b", d=2, m=2)
            po_r = po[:].rearrange("p (d m b) -> p d m b", d=2, m=2)
            # IF group
            nc.tensor.matmul(out=pif[:], lhsT=ident_bf[:],
                             rhs=gx_p[:, 0:8, t, :], start=True, stop=False)
            for d in range(2):
                for mi in range(4):
                    for kc in range(2):
                        nc.tensor.matmul(
                            out=pif_r[:, d, mi, :], lhsT=whh_r[:, d, kc, mi, :],
                            rhs=hall_r[:, d, kc, t, :],
                            start=False, stop=(kc == 1))
            # G group
            nc.tensor.matmul(out=pg[:], lhsT=ident_bf[:],
                             rhs=gx_p[:, 8:12, t, :], start=True, stop=False)
            for d in range(2):
                for mi in range(2):
                    for kc in range(2):
                        nc.tensor.matmul(
                            out=pg_r[:, d, mi, :], lhsT=whh_r[:, d, kc, 6 + mi, :],
                            rhs=hall_r[:, d, kc, t, :],
                            start=False, stop=(kc == 1))
            # O group
            nc.tensor.matmul(out=po[:], lhsT=ident_bf[:],
                             rhs=gx_p[:, 12:16, t, :], start=True, stop=False)
            for d in range(2):
                for mi in range(2):
                    for kc in range(2):
                        nc.tensor.matmul(
                            out=po_r[:, d, mi, :], lhsT=whh_r[:, d, kc, 4 + mi, :],
                            rhs=hall_r[:, d, kc, t, :],
                            start=False, stop=(kc == 1))

            sigIF = scratch.tile([128, 64], F32, tag="sigIF")
            sif_r = sigIF[:].rearrange("p (d m b) -> p d m b", d=2, m=4)
            nc.scalar.activation(out=sigIF[:], in_=pif[:], func=SIG)
            tg = scratch.tile([128, 32], F32, tag="tg")
            nc.scalar.activation(out=tg[:], in_=pg[:], func=TANH)
            so = scratch.tile([128, 32], F32, tag="so")
            nc.scalar.activation(out=so[:], in_=po[:], func=SIG)

            cf = scratch.tile([128, 32], F32, tag="cf")
            cf_r = cf[:].rearrange("p (d c b) -> p d c b", d=2, c=2)
            nc.vector.tensor_mul(out=cf_r[:, :, :, :], in0=sif_r[:, :, 2:4, :],
                                 in1=cst_r[:, :, :, :])
            t1 = scratch.tile([128, 32], F32, tag="t1")
            t1_r = t1[:].rearrange("p (d c b) -> p d c b", d=2, c=2)
            nc.vector.tensor_mul(out=t1_r[:, :, :, :], in0=sif_r[:, :, 0:2, :],
                                 in1=tg[:].rearrange("p (d c b) -> p d c b", d=2, c=2))
            nc.vector.tensor_add(out=cst[:], in0=cf[:], in1=t1[:])
            tc_ = scratch.tile([128, 32], F32, tag="tc")
            nc.scalar.activation(out=tc_[:], in_=cst[:], func=TANH)
            nc.vector.tensor_mul(
                out=hall_r[:, :, :, t + 1, :],
                in0=so[:].rearrange("p (d c b) -> p d c b", d=2, c=2),
                in1=tc_[:].rearrange("p (d c b) -> p d c b", d=2, c=2))

            # hb bounce-out rides the idle TE/VE/DMA slots of the scan
            if t % 16 == 15:
                s_ = t // 16
                hbs = gather_p.tile([128, H], BF16, tag="hbs")
                for ec in range(2):
                    pt = ps_hb.tile([128, 128], BF16, tag="pth")
                    nc.tensor.transpose(
                        out=pt[:],
                        in_=hall_r[:, 1, ec, 1 + s_ * 16:1 + (s_ + 1) * 16, :],
                        identity=ident_bf[:])
                    nc.vector.tensor_copy(out=hbs[:, ec * 128:(ec + 1) * 128],
                                          in_=pt[:])
                nc.sync.dma_start(out=hb_dram[s_ * 128:(s_ + 1) * 128, :],
                                  in_=hbs[:])

        gx_ctx.close()
        scan_ctx.close()

        # ---- P4: hb re-reversal (DRAM bounce + indirect gather + transpose),
        #          then fc emissions; all (t, b) t-major ----
        p4_ctx = ExitStack()
        ps_tr = p4_ctx.enter_context(tc.tile_pool(name="ps_tr2", bufs=2, space="PSUM"))
        ps_fc = p4_ctx.enter_context(tc.tile_pool(name="ps_fc", bufs=2, space="PSUM"))
        hbT = state.tile([128, 2 * NBT], BF16)   # [p(hid), kc, bt']
        hbT_r = hbT[:].rearrange("p (kc n) -> p kc n", kc=2)
        for s_ in range(NSLOT):
            hs = gather_p.tile([128, H], BF16, tag="hslot")
            nc.gpsimd.indirect_dma_start(
                out=hs[:], out_offset=None, in_=hb_dram,
                in_offset=IndirectOffsetOnAxis(ap=hboff[:, s_:s_ + 1], axis=0))
            for ec in range(2):
                pt = ps_tr.tile([128, 128], BF16, tag="ptr")
                nc.tensor.transpose(out=pt[:], in_=hs[:, ec * 128:(ec + 1) * 128],
                                    identity=ident_bf[:])
                nc.vector.tensor_copy(out=hbT_r[:, ec, s_ * 128:(s_ + 1) * 128], in_=pt[:])

        feats_sb = state.tile([128, 16 * K], F32)   # [p=(tw,b), mt, k]
        feats_r = feats_sb[:].rearrange("p (m k) -> p m k", m=16)
        for mt in range(16):
            pf = ps_fc.tile([128, K], F32, tag="pfc")
            for c4 in range(4):
                if c4 < 2:
                    lhs = hall_r[:, 0, c4, 1 + mt * 16:1 + (mt + 1) * 16, :]
                else:
                    lhs = hbT_r[:, c4 - 2, mt * 128:(mt + 1) * 128]
                nc.tensor.matmul(out=pf[:], lhsT=lhs, rhs=fcw_r[:, c4, :],
                                 start=(c4 == 0), stop=(c4 == 3))
            nc.vector.tensor_tensor(out=feats_r[:, mt, :], in0=pf[:],
                                    in1=fcbR[:, :], op=AL.add)
        p4_ctx.close()

        # relayout feats -> [b partitions, (t, k)]; p=(tw, b), t = mt*16+tw
        # bounce stored (tw, b, mt, k); load permutes to (b, mt, tw, k) = (b, t, k)
        nc.sync.dma_start(
            out=feats_dram.rearrange("(tw b mt k) -> (tw b) mt k", tw=16, b=BL, mt=16),
            in_=feats_r[:, :, :])
        feats8 = state.tile([128, T * K], F32)
        nc.sync.dma_start(
            out=feats8[0:BL, :].rearrange("p (mt tw k) -> p mt tw k", mt=16, tw=16),
            in_=feats_dram.rearrange("(tw b mt k) -> b mt tw k", tw=16, b=BL, mt=16))
        f8_r = feats8[:].rearrange("p (t k) -> p t k", t=T)

        # ==== P5: Viterbi forward scan (exact serial, batch on partitions) ====
        vit_p = ctx.enter_context(tc.tile_pool(name="vit", bufs=4))
        vbig = ctx.enter_context(tc.tile_pool(name="vbig", bufs=1))
        preH = state.tile([128, T * K], F32)
        preH_r = preH[:].rearrange("p (t k) -> p t k", t=T)
        nc.vector.tensor_copy(out=preH_r[0:BL, 0, :], in_=f8_r[0:BL, 0, :])
        for t in range(1, T):
            s1 = vit_p.tile([128, K * K], F32, tag="s1")
            nc.vector.tensor_tensor(
                out=s1[0:BL, :].rearrange("p (c q) -> p c q", c=K),
                in0=preH_r[0:BL, t - 1, :].unsqueeze(1)
                    .broadcast_to((BL, K, K)),
                in1=transR[0:BL, :].rearrange("p (c q) -> p c q", c=K), op=AL.add)
            m1 = vit_p.tile([128, K], F32, tag="m1")
            nc.vector.tensor_reduce(
                out=m1[0:BL, :], in_=s1[0:BL, :].rearrange("p (c q) -> p c q", c=K),
                axis=AX.X, op=AL.max)
            nc.vector.tensor_tensor(out=preH_r[0:BL, t, :], in0=m1[0:BL, :],
                                    in1=f8_r[0:BL, t, :], op=AL.add)

        # ==== P6: batched backpointer extraction (exact) ====
        nc.sync.dma_start(
            out=pre_dram.rearrange("(t b k) -> b t k", t=T, b=BL),
            in_=preH_r[0:BL, :, :])
        preT = vbig.tile([128, 2 * BL * K], F32, tag="preT")
        preT_r = preT[:].rearrange("p (h b k) -> p h b k", h=2, b=BL)
        nc.sync.dma_start(
            out=preT_r[:, :, :, :],
            in_=pre_dram.rearrange("(h p b k) -> p h b k", h=2, p=128, b=BL))

        HB = 2 * BL
        preT_hb = preT[:].rearrange("p (hb k) -> p hb k", k=K)
        sX = vbig.tile([128, 2 * BL * K * K], F32, tag="sX")
        sX_r = sX[:].rearrange("p (hb c q) -> p hb c q", hb=HB, c=K)
        nc.vector.tensor_tensor(
            out=sX_r[:, :, :, :],
            in0=preT_hb.unsqueeze(2).broadcast_to((128, HB, K, K)),
            in1=transR[:, :].rearrange("p (c q) -> p c q", c=K).unsqueeze(1)
                .broadcast_to((128, HB, K, K)),
            op=AL.add)
        mX = vbig.tile([128, 2 * BL * K], F32, tag="mX")
        mX_r = mX[:].rearrange("p (hb c) -> p hb c", hb=HB)
        nc.vector.tensor_reduce(out=mX_r[:, :, :], in_=sX_r[:, :, :, :],
                                axis=AX.X, op=AL.max)
        nc.vector.tensor_tensor(
            out=sX_r[:, :, :, :], in0=sX_r[:, :, :, :],
            in1=mX_r[:, :, :].unsqueeze(3).broadcast_to((128, HB, K, K)),
            op=AL.is_equal)
        nc.vector.tensor_tensor(
            out=sX_r[:, :, :, :], in0=sX_r[:, :, :, :],
            in1=iotaD[:, :].unsqueeze(1).unsqueeze(1).broadcast_to((128, HB, K, K)),
            op=AL.mult)
        bq = vbig.tile([128, 2 * BL * K], F32, tag="bq")
        bq_r = bq[:].rearrange("p (hb c) -> p hb c", hb=HB)
        nc.vector.tensor_reduce(out=bq_r[:, :, :], in_=sX_r[:, :, :, :],
                                axis=AX.X, op=AL.max)
        # bp = 9 - bq ; then pad override: bp*valid + iota_cur*(1-valid)
        nc.vector.tensor_scalar(out=bq[:], in0=bq[:], scalar1=-1.0, scalar2=9.0,
                                op0=AL.mult, op1=AL.add)
        nc.vector.tensor_tensor(
            out=bq_r[:, :, :], in0=bq_r[:, :, :],
            in1=validT[:, :].unsqueeze(2).broadcast_to((128, HB, K)),
            op=AL.mult)
        nc.vector.tensor_tensor(
            out=bq_r[:, :, :], in0=bq_r[:, :, :],
            in1=ivT[:, :].rearrange("p (hb k) -> p hb k", k=K),
            op=AL.add)
        # store bp (bq row t_idx = bp at time t_idx+1) into flat (b, t, k)
        # order shifted forward by one step via a K-element front pad; the
        # shifted reload then yields bpQ[(b,grp), (tw, c)] = m_t (bp at t),
        # with t=0 slots landing on iota (identity), as the backtrace wants.
        iK = vit_p.tile([128, K], F32, tag="iK")
        nc.vector.tensor_copy(out=iK[:], in_=iotaK[:, :])
        nc.sync.dma_start(out=bp2dram[0:K].rearrange("(o x) -> o x", o=1),
                          in_=iK[0:1, :])
        bq4 = bq[:].rearrange("p (h b k) -> p h b k", h=2, b=BL)
        bp2v = bp2dram[K:].rearrange("(b h2 p k) -> h2 p b k", b=BL, h2=2, p=128)
        for h_ in range(2):
            nc.sync.dma_start(out=bp2v[h_], in_=bq4[:, h_, :, :])
        bpQ = vbig.tile([128, 160], F32, tag="bpQ")
        nc.sync.dma_start(out=bpQ[:],
                          in_=bp2dram[0:128 * 160].rearrange("(p x) -> p x", p=128))

        # ==== end-tag (exact, from preH) ====
        pesel = vbig.tile([128, T * K], F32, tag="pesel")
        nc.vector.tensor_tensor(
            out=pesel[0:BL, :].rearrange("p (t k) -> p t k", t=T),
            in0=preH_r[0:BL, :, :],
            in1=endSel[0:BL, :].unsqueeze(2).broadcast_to((BL, T, K)),
            op=AL.mult)
        pe = vbig.tile([128, K], F32, tag="pe")
        nc.vector.tensor_reduce(
            out=pe[0:BL, :],
            in_=pesel[0:BL, :].rearrange("p (t k) -> p k t", t=T),
            axis=AX.X, op=AL.max)
        mvE = vit_p.tile([128, 1], F32, tag="mvE")
        nc.vector.tensor_reduce(out=mvE[0:BL, :], in_=pe[0:BL, :], axis=AX.X, op=AL.max)
        eqE = vit_p.tile([128, K], F32, tag="eqE")
        nc.vector.tensor_tensor(out=eqE[0:BL, :], in0=pe[0:BL, :],
                                in1=mvE[0:BL, :].broadcast_to((BL, K)), op=AL.is_equal)
        nc.vector.tensor_mul(out=eqE[0:BL, :], in0=eqE[0:BL, :], in1=iotaD[0:BL, :])
        endT = vit_p.tile([128, 1], F32, tag="endT")
        nc.vector.tensor_reduce(out=endT[0:BL, :], in_=eqE[0:BL, :], axis=AX.X, op=AL.max)
        nc.vector.tensor_scalar(out=endT[0:BL, :], in0=endT[0:BL, :],
                                scalar1=-1.0, scalar2=9.0, op0=AL.mult, op1=AL.add)

        # ==== backtrace via map composition ====
        bpQ4 = bpQ[:].rearrange("p (i w k) -> p i w k", i=4, w=4)

        def compose4(a_view, b_view, tag):
            # out[p, i, k] = a[p, i, b[p, i, k]] for 4 quads per partition
            e4 = vit_p.tile([128, 400], F32, tag="e4")
            e4_r = e4[:].rearrange("p (i k j) -> p i k j", i=4, k=K)
            nc.vector.tensor_tensor(
                out=e4_r, in0=b_view.unsqueeze(3).broadcast_to((128, 4, K, K)),
                in1=iotaK[:, :].unsqueeze(1).unsqueeze(1)
                    .broadcast_to((128, 4, K, K)),
                op=AL.is_equal)
            nc.vector.tensor_tensor(
                out=e4_r, in0=e4_r,
                in1=a_view.unsqueeze(2).broadcast_to((128, 4, K, K)), op=AL.mult)
            o = vbig.tile([128, 4 * K], F32, tag=tag)
            o_r = o[:].rearrange("p (i k) -> p i k", i=4)
            nc.vector.tensor_reduce(out=o_r, in_=e4_r, axis=AX.X, op=AL.max)
            return o, o_r

        sk3 = bpQ4[:, :, 3, :]
        sk2, sk2_r = compose4(bpQ4[:, :, 2, :], sk3, "sk2")
        sk1, sk1_r = compose4(bpQ4[:, :, 1, :], sk2_r, "sk1")
        sk0, sk0_r = compose4(bpQ4[:, :, 0, :], sk1_r, "sk0")   # Fq per quad

        def compose1(a_view, b_view, tag):
            # out[p, k] = a[p, b[p, k]]
            e1 = vit_p.tile([128, K * K], F32, tag="e1")
            e1_r = e1[:].rearrange("p (k j) -> p k j", k=K)
            nc.vector.tensor_tensor(
                out=e1_r, in0=b_view.unsqueeze(2).broadcast_to((128, K, K)),
                in1=iotaK[:, :].unsqueeze(1).broadcast_to((128, K, K)),
                op=AL.is_equal)
            nc.vector.tensor_tensor(
                out=e1_r, in0=e1_r,
                in1=a_view.unsqueeze(1).broadcast_to((128, K, K)), op=AL.mult)
            o = vbig.tile([128, K], F32, tag=tag)
            nc.vector.tensor_reduce(out=o[:], in_=e1_r, axis=AX.X, op=AL.max)
            return o

        sg3 = sk0_r[:, 3, :]
        sg2 = compose1(sk0_r[:, 2, :], sg3, "sg2")
        sg1 = compose1(sk0_r[:, 1, :], sg2[:], "sg1")
        sg0 = compose1(sk0_r[:, 0, :], sg1[:], "sg0")           # G per grp

        # G relayout -> [b, (grp, k)]
        nc.sync.dma_start(out=g2dram.rearrange("(p x) -> p x", p=128), in_=sg0[:])
        Gs = vbig.tile([128, 16 * K], F32, tag="Gs")
        nc.sync.dma_start(out=Gs[0:BL, :], in_=g2dram.rearrange("(b x) -> b x", b=BL))
        Gs_r = Gs[:].rearrange("b (g k) -> b g k", g=16)

        # serial grp chase: bnd_15 = end; bnd_{g-1} = G_g[bnd_g]
        bndAll = vbig.tile([128, 16], F32, tag="bndAll")
        nc.vector.tensor_copy(out=bndAll[0:BL, 15:16], in_=endT[0:BL, :])
        for g in range(15, 0, -1):
            eqA = vit_p.tile([128, K], F32, tag="eqA")
            nc.vector.tensor_tensor(
                out=eqA[0:BL, :], in0=iotaK[0:BL, :],
                in1=bndAll[0:BL, g:g + 1].broadcast_to((BL, K)), op=AL.is_equal)
            nc.vector.tensor_mul(out=eqA[0:BL, :], in0=eqA[0:BL, :], in1=Gs_r[0:BL, g, :])
            nc.vector.tensor_reduce(out=bndAll[0:BL, g - 1:g], in_=eqA[0:BL, :],
                                    axis=AX.X, op=AL.max)

        # bnd relayout -> [(b,grp), 1]
        nc.sync.dma_start(out=bnddram.rearrange("(b x) -> b x", b=BL), in_=bndAll[0:BL, :])
        bndP = vbig.tile([128, 1], F32, tag="bndP")
        nc.sync.dma_start(out=bndP[:], in_=bnddram.rearrange("(p x) -> p x", p=128))

        # sgq[p, i, j]: entry map for quad i = sg_{i+1} (sg_4 = Id)
        sgq = vbig.tile([128, 4 * K], F32, tag="sgq")
        nc.vector.tensor_copy(out=sgq[:, 0:K], in_=sg1[:])
        nc.vector.tensor_copy(out=sgq[:, K:2 * K], in_=sg2[:])
        nc.vector.tensor_copy(out=sgq[:, 2 * K:3 * K], in_=sk0_r[:, 3, :])
        nc.vector.tensor_copy(out=sgq[:, 3 * K:4 * K], in_=iotaK[:, :])
        # qbEntry[p, i] = sgq_i[bnd]
        eqi = vit_p.tile([128, 4 * K], F32, tag="eqi")
        eqi_r = eqi[:].rearrange("p (i j) -> p i j", i=4)
        nc.vector.tensor_tensor(
            out=eqi_r,
            in0=iotaK[:, :].unsqueeze(1).broadcast_to((128, 4, K)),
            in1=bndP[:].unsqueeze(1).broadcast_to((128, 4, K)), op=AL.is_equal)
        nc.vector.tensor_tensor(
            out=eqi_r, in0=eqi_r,
            in1=sgq[:].rearrange("p (i j) -> p i j", i=4), op=AL.mult)
        qbE = vbig.tile([128, 4], F32, tag="qbE")
        nc.vector.tensor_reduce(out=qbE[:], in_=eqi_r, axis=AX.X, op=AL.max)

        # skq[p, i, w, j]: within-quad entry maps = sk_{w+1} (sk_4 = Id)
        skq = vbig.tile([128, 160], F32, tag="skq")
        skq_r = skq[:].rearrange("p (i w j) -> p i w j", i=4, w=4)
        nc.vector.tensor_copy(out=skq_r[:, :, 0, :], in_=sk1_r)
        nc.vector.tensor_copy(out=skq_r[:, :, 1, :], in_=sk2_r)
        nc.vector.tensor_copy(out=skq_r[:, :, 2, :], in_=bpQ4[:, :, 3, :])
        for _i in range(4):
            nc.vector.tensor_copy(out=skq_r[:, _i, 3, :], in_=iotaK[:, :])
        # tagOut[p, (i, w)] = skq_{i,w}[qbE_i]
        eqt = vit_p.tile([128, 160], F32, tag="eqt")
        eqt_r = eqt[:].rearrange("p (i w j) -> p i w j", i=4, w=4)
        nc.vector.tensor_tensor(
            out=eqt_r,
            in0=iotaK[:, :].unsqueeze(1).unsqueeze(1).broadcast_to((128, 4, 4, K)),
            in1=qbE[:].rearrange("p (i o) -> p i o", i=4).unsqueeze(3)
                .broadcast_to((128, 4, 4, K)),
            op=AL.is_equal)
        nc.vector.tensor_tensor(out=eqt_r, in0=eqt_r, in1=skq_r, op=AL.mult)
        tagOut = vbig.tile([128, 16], F32, tag="tagOut")
        nc.vector.tensor_reduce(out=tagOut[:].rearrange("p (i w) -> p i w", i=4),
                                in_=eqt_r, axis=AX.X, op=AL.max)

        bestQ = vbig.tile([128, 16], F32, tag="bestQ")
        nc.vector.tensor_mul(out=bestQ[:], in0=tagOut[:], in1=maskQ[:, :])
        nc.sync.dma_start(
            out=out_d.rearrange("b (grp tw) -> (b grp) tw", grp=16),
            in_=bestQ[:])
        ctx.close()

    nc.compile()
    return nc


_NC_CACHE = None


def _get_nc():
    global _NC_CACHE
    if _NC_CACHE is None:
        _NC_CACHE = _build()
    return _NC_CACHE


TRACE = False
LAST_EXEC_NS = None


def kernel(**inputs) -> np.ndarray:
    global LAST_EXEC_NS
    nc = _get_nc()
    in_maps = [_prep_core(inputs, c) for c in range(NC_)]
    res = run_bass_kernel_spmd(nc, in_maps, list(range(NC_)), trace=TRACE)
    LAST_EXEC_NS = res.exec_time_ns
    out = np.concatenate([res.results[c]['out'] for c in range(NC_)], axis=0)
    return out.astype(np.float32)


if __name__ == '__main__':
    _build()
    print("build ok")
